# revision 7
# baseline (speedup 1.0000x reference)
"""AttentionBlock (GroupNorm -> QKV -> full attention -> out-proj + residual)
for B=4, C=128, N=4096 on 8 Trainium2 NeuronCores.

Sharding: 8 cores = 4 batches x 2 query-slabs of N/2. Every core runs the
same program; the host rolls each core's x so its query slab is always
columns [0, N/2).

Matmuls run in float32r (fp32 data, PE rounds to ~tf32 -> ~1e-4 rel err at
bf16 speed). Scores are computed transposed [j, i] so softmax's exp feeds
the PV matmul with no transposes; row sums come from an all-ones matmul
accumulated alongside PV, and the normalization is one multiply by a
DMA-broadcast reciprocal at the end.
"""

import math
import sys

if "/opt/trn_rl_repo" not in sys.path:
    sys.path.insert(0, "/opt/trn_rl_repo")

import numpy as np

C = 128
G = 8
GS = C // G  # channels per group
EPS = 1e-5
N_CORES = 8


def build(N=4096):
    """Build the per-core Bass program. Returns the compiled Bacc module."""
    import concourse.bacc as bacc
    import concourse.bass as bass
    import concourse.mybir as mybir
    import concourse.tile as tile

    f32 = mybir.dt.float32
    f32r = mybir.dt.float32r
    AF = mybir.ActivationFunctionType
    OP = mybir.AluOpType

    S = N // 2           # query slab width per core
    ICW = min(1024, S)   # i-chunk width (one PV/rowsum accumulation pass)
    NIC = S // ICW       # number of i-chunk passes
    SC = min(512, ICW)   # score chunk width (one QK matmul / exp op)
    NSC = ICW // SC      # score chunks per i-chunk
    NJT = N // 128       # number of j (key) tiles
    BNC = min(512, N)    # bn_stats chunk
    NBN = N // BNC
    PCW = min(512, S)    # projection/epilogue chunk width for slab-sized tensors
    NPC = S // PCW
    SCALE = 1.0 / math.sqrt(C)

    nc = bacc.Bacc("TRN2", target_bir_lowering=False, debug=False)

    x_d = nc.dram_tensor("x", [C, N], f32, kind="ExternalInput").ap()
    w_d = nc.dram_tensor("wcat", [C, 4 * C], f32, kind="ExternalInput").ap()
    m_d = nc.dram_tensor("gmask", [C, C], f32, kind="ExternalInput").ap()
    b_d = nc.dram_tensor("bcat", [C, 5], f32, kind="ExternalInput").ap()
    o_d = nc.dram_tensor("out", [C, S], f32, kind="ExternalOutput").ap()
    # DRAM scratch for the rowsum reshape/broadcast round-trips
    rs1_d = nc.dram_tensor("rs_scr1", [NIC, ICW], f32).ap()
    rs2_d = nc.dram_tensor("rs_scr2", [NIC, ICW], f32).ap()

    with tile.TileContext(nc) as tc:
        with tc.tile_pool(name="consts", bufs=1) as cp, \
             tc.tile_pool(name="big", bufs=1) as bp, \
             tc.tile_pool(name="small", bufs=2) as sp_, \
             tc.tile_pool(name="pP", bufs=4) as pP:

            # ---- loads + constants ----
            xS = bp.tile([C, N], f32, tag="x")
            nc.sync.dma_start(xS[:], x_d[:])
            wS = cp.tile([C, 4 * C], f32, tag="w")
            nc.sync.dma_start(wS[:], w_d[:])
            wR = cp.tile([C, 4 * C], f32r, tag="wr")
            nc.vector.tensor_copy(wR[:], wS[:])
            mS = cp.tile([C, C], f32, tag="gmask")
            nc.sync.dma_start(mS[:], m_d[:])
            bS = cp.tile([C, 5], f32, tag="bcat")
            nc.sync.dma_start(bS[:], b_d[:])
            onesS = cp.tile([C, C], f32, tag="ones")
            nc.vector.memset(onesS[:], 1.0)
            onesR = cp.tile([C, C], f32r, tag="onesr")
            nc.vector.tensor_copy(onesR[:], onesS[:])
            epsT = cp.tile([C, 1], f32, tag="eps")
            nc.vector.memset(epsT[:], EPS)

            hR = bp.tile([C, N], f32r, tag="h")
            kR = bp.tile([C, N], f32r, tag="k")
            qR = bp.tile([C, S], f32r, tag="q")
            vTR = bp.tile([C, N], f32r, tag="vT")
            h2nR = bp.tile([C, S], f32r, tag="h2n")
            outS = bp.tile([C, S], f32, tag="outS")

            with tc.tile_pool(name="ps_pre", bufs=2, space="PSUM") as pre:
                # ---- GroupNorm stats ----
                st6 = sp_.tile([C, NBN, 6], f32, tag="st6")
                for i in range(NBN):
                    nc.vector.bn_stats(out=st6[:, i, :], in_=xS[:, i * BNC:(i + 1) * BNC])
                mv = sp_.tile([C, 2], f32, tag="mv")
                nc.vector.bn_aggr(out=mv[:], in_=st6[:])
                # stats2 = [mean, E[x^2]] per channel
                st2 = sp_.tile([C, 2], f32, tag="st2")
                nc.vector.tensor_copy(st2[:, 0:1], mv[:, 0:1])
                nc.vector.tensor_tensor(out=st2[:, 1:2], in0=mv[:, 0:1], in1=mv[:, 0:1], op=OP.mult)
                nc.vector.tensor_tensor(out=st2[:, 1:2], in0=st2[:, 1:2], in1=mv[:, 1:2], op=OP.add)
                # cross-partition group reduce: gstats[c,:] = [gmean, gEx2] of c's group
                gps = pre.tile([C, 2], f32, tag="gstats")
                nc.tensor.matmul(gps[:], mS[:], st2[:], start=True, stop=True)
                gst = sp_.tile([C, 2], f32, tag="gst")
                nc.vector.tensor_copy(gst[:], gps[:])
                gv = sp_.tile([C, 1], f32, tag="gv")
                nc.vector.tensor_tensor(out=gv[:], in0=gst[:, 0:1], in1=gst[:, 0:1], op=OP.mult)
                nc.vector.tensor_tensor(out=gv[:], in0=gst[:, 1:2], in1=gv[:], op=OP.subtract)
                sd = sp_.tile([C, 1], f32, tag="sd")
                nc.scalar.activation(out=sd[:], in_=gv[:], func=AF.Sqrt, bias=epsT[:], scale=1.0)
                inv = sp_.tile([C, 1], f32, tag="inv")
                nc.vector.reciprocal(out=inv[:], in_=sd[:])
                aT = sp_.tile([C, 1], f32, tag="aT")
                nc.vector.tensor_tensor(out=aT[:], in0=bS[:, 3:4], in1=inv[:], op=OP.mult)
                bT = sp_.tile([C, 1], f32, tag="bT")
                nc.vector.tensor_tensor(out=bT[:], in0=gst[:, 0:1], in1=aT[:], op=OP.mult)
                nc.vector.tensor_tensor(out=bT[:], in0=bS[:, 4:5], in1=bT[:], op=OP.subtract)
                # h = a*x + b  (rounded to f32r)
                nc.vector.tensor_scalar(out=hR[:], in0=xS[:], scalar1=aT[:], scalar2=bT[:],
                                        op0=OP.mult, op1=OP.add)

                # ---- projections ----
                for c in range(N // 512):
                    kp = pre.tile([C, 512], f32, tag="kp")
                    nc.tensor.matmul(kp[:], wR[:, C:2 * C], hR[:, 512 * c:512 * (c + 1)],
                                     start=True, stop=True)
                    nc.vector.tensor_scalar(out=kR[:, 512 * c:512 * (c + 1)], in0=kp[:],
                                            scalar1=bS[:, 1:2], scalar2=None, op0=OP.add)
                for c in range(NPC):
                    qp = pre.tile([C, PCW], f32, tag="qp")
                    nc.tensor.matmul(qp[:], wR[:, 0:C], hR[:, PCW * c:PCW * (c + 1)],
                                     start=True, stop=True)
                    nc.vector.tensor_scalar(out=qR[:, PCW * c:PCW * (c + 1)], in0=qp[:],
                                            scalar1=bS[:, 0:1], scalar2=None, op0=OP.add)
                for g in range(N // 512):
                    vtp = pre.tile([C, 512], f32, tag="vtp")
                    for j4 in range(4):
                        t = 4 * g + j4
                        nc.tensor.matmul(vtp[:, 128 * j4:128 * (j4 + 1)],
                                         hR[:, 128 * t:128 * (t + 1)], wR[:, 2 * C:3 * C],
                                         start=True, stop=True)
                    nc.vector.tensor_copy(vTR[:, 512 * g:512 * (g + 1)], vtp[:])

            # ---- attention ----
            h2ps = []
            with tc.tile_pool(name="ps_sT", bufs=2, space="PSUM") as psT, \
                 tc.tile_pool(name="ps_h2", bufs=2, space="PSUM") as ph2, \
                 tc.tile_pool(name="ps_rs", bufs=1, space="PSUM") as prs:
                for ic in range(NIC):
                    h2p = ph2.tile([C, ICW], f32, tag="h2u")
                    h2ps.append(h2p)
                    rsp = prs.tile([C, ICW], f32, tag="rs")
                    for t in range(NJT):
                        for c2 in range(NSC):
                            i0 = ic * ICW + c2 * SC
                            sT = psT.tile([C, SC], f32, tag="sT")
                            nc.tensor.matmul(sT[:], kR[:, 128 * t:128 * (t + 1)],
                                             qR[:, i0:i0 + SC], start=True, stop=True)
                            Pt = pP.tile([C, SC], f32r, tag="P")
                            nc.scalar.activation(out=Pt[:], in_=sT[:], func=AF.Exp, scale=SCALE)
                            nc.tensor.matmul(h2p[:, c2 * SC:(c2 + 1) * SC],
                                             vTR[:, 128 * t:128 * (t + 1)], Pt[:],
                                             start=(t == 0), stop=(t == NJT - 1))
                            nc.tensor.matmul(rsp[:, c2 * SC:(c2 + 1) * SC],
                                             onesR[:], Pt[:],
                                             start=(t == 0), stop=(t == NJT - 1))
                    # rowsum (replicated on all partitions) -> SBUF -> DRAM
                    rsS = sp_.tile([1, ICW], f32, tag="rsS")
                    nc.vector.tensor_copy(rsS[:], rsp[0:1, :])
                    nc.sync.dma_start(rs1_d[ic:ic + 1, :], rsS[:])

                # normalization: h2n = h2 * (1/rowsum), broadcast along partitions
                for ic in range(NIC):
                    r8 = sp_.tile([C, ICW // 128], f32, tag="r8")
                    nc.sync.dma_start(
                        r8[:], rs1_d[ic:ic + 1, :].rearrange("a (p f) -> (a p) f", p=128))
                    r8i = sp_.tile([C, ICW // 128], f32, tag="r8i")
                    nc.vector.reciprocal(out=r8i[:], in_=r8[:])
                    nc.sync.dma_start(
                        rs2_d[ic:ic + 1, :].rearrange("a (p f) -> (a p) f", p=128), r8i[:])
                    row = rs2_d[ic:ic + 1, :]
                    bcast = bass.AP(tensor=row.tensor, offset=row.offset,
                                    ap=[[0, C], row.ap[-1]])
                    recipB = sp_.tile([C, ICW], f32, tag="recipB")
                    nc.sync.dma_start(recipB[:], bcast)
                    nc.vector.tensor_tensor(out=h2nR[:, ic * ICW:(ic + 1) * ICW],
                                            in0=h2ps[ic][:], in1=recipB[:], op=OP.mult)

            # ---- out projection + bias + residual ----
            with tc.tile_pool(name="ps_ep", bufs=2, space="PSUM") as pep:
                for c in range(NPC):
                    pop = pep.tile([C, PCW], f32, tag="pop")
                    nc.tensor.matmul(pop[:], wR[:, 3 * C:4 * C],
                                     h2nR[:, PCW * c:PCW * (c + 1)], start=True, stop=True)
                    nc.vector.scalar_tensor_tensor(
                        out=outS[:, PCW * c:PCW * (c + 1)], in0=pop[:], scalar=bS[:, 2:3],
                        in1=xS[:, PCW * c:PCW * (c + 1)], op0=OP.add, op1=OP.add)
            nc.sync.dma_start(o_d[:], outS[:])

    nc.compile()
    return nc


def host_inputs(x, gn_w, gn_b, w_qkv, b_qkv, w_out, b_out):
    """Build the 8 per-core input maps from the full problem inputs."""
    x = np.asarray(x, dtype=np.float32)
    B, _, N = x.shape
    S = N // 2
    w_qkv = np.asarray(w_qkv, np.float32)
    w_out = np.asarray(w_out, np.float32)
    b_qkv = np.asarray(b_qkv, np.float32)
    b_out = np.asarray(b_out, np.float32)
    gn_w = np.asarray(gn_w, np.float32)
    gn_b = np.asarray(gn_b, np.float32)

    wcat = np.concatenate(
        [w_qkv[0:C].T, w_qkv[C:2 * C].T, w_qkv[2 * C:3 * C].T, w_out.T],
        axis=1).astype(np.float32)                      # [C, 4C], each [c_in, c_out]
    gidx = np.arange(C) // GS
    gmask = (gidx[:, None] == gidx[None, :]).astype(np.float32) / GS
    b_eff = b_out + w_out @ b_qkv[2 * C:3 * C]
    bcat = np.stack([b_qkv[0:C], b_qkv[C:2 * C], b_eff, gn_w, gn_b], axis=1)
    bcat = np.ascontiguousarray(bcat, np.float32)       # [C, 5]

    in_maps = []
    for core in range(N_CORES):
        b, half = divmod(core, 2)
        xb = np.roll(x[b], -half * S, axis=1)
        in_maps.append({"x": np.ascontiguousarray(xb), "wcat": wcat,
                        "gmask": gmask, "bcat": bcat})
    return in_maps


_NC_CACHE = {}
_RUNNER_CACHE = {}


def _make_runner(nc):
    """Compile-once runner: replicates bass2jax.run_bass_via_pjrt but keeps the
    jitted sharded callable so repeat executions skip recompilation."""
    import jax
    import concourse.mybir as mybir
    from jax.sharding import Mesh, PartitionSpec
    from jax.experimental.shard_map import shard_map
    from concourse.bass2jax import (_bass_exec_p, install_neuronx_cc_hook,
                                    partition_id_tensor)

    install_neuronx_cc_hook()
    partition_name = nc.partition_id_tensor.name if nc.partition_id_tensor else None
    in_names, out_names, out_avals, zero_shapes = [], [], [], []
    for alloc in nc.m.functions[0].allocations:
        if not isinstance(alloc, mybir.MemoryLocationSet):
            continue
        name = alloc.memorylocations[0].name
        if alloc.kind == "ExternalInput":
            if name == partition_name:
                continue
            in_names.append(name)
        elif alloc.kind == "ExternalOutput":
            out_names.append(name)
            shape = tuple(alloc.tensor_shape)
            dtype = mybir.dt.np(alloc.dtype)
            out_avals.append(jax.core.ShapedArray(shape, dtype))
            zero_shapes.append((shape, dtype))
    n_params = len(in_names)
    all_names = in_names + out_names
    if partition_name is not None:
        all_names = all_names + [partition_name]
    donate = tuple(range(n_params, n_params + len(out_names)))

    def _body(*args):
        operands = list(args)
        if partition_name is not None:
            operands.append(partition_id_tensor())
        return tuple(_bass_exec_p.bind(
            *operands, out_avals=tuple(out_avals), in_names=tuple(all_names),
            out_names=tuple(out_names), lowering_input_output_aliases=(),
            sim_require_finite=True, sim_require_nnan=True, nc=nc))

    devices = jax.devices()[:N_CORES]
    mesh = Mesh(np.asarray(devices), ("core",))
    specs = (PartitionSpec("core"),)
    sharded = jax.jit(
        shard_map(_body, mesh=mesh,
                  in_specs=specs * (n_params + len(out_names)),
                  out_specs=specs * len(out_names), check_rep=False),
        donate_argnums=donate, keep_unused=True)

    def run(in_maps):
        concat_in = [np.concatenate([np.asarray(m[nm]) for m in in_maps], axis=0)
                     for nm in in_names]
        concat_zeros = [np.zeros((N_CORES * s[0], *s[1:]), d) for s, d in zero_shapes]
        out_arrs = sharded(*concat_in, *concat_zeros)
        out_arrs = [np.asarray(a) for a in out_arrs]
        return [{nm: out_arrs[i].reshape(N_CORES, *out_avals[i].shape)[c]
                 for i, nm in enumerate(out_names)} for c in range(N_CORES)]

    return run


def get_runner(N=4096):
    if N not in _RUNNER_CACHE:
        if N not in _NC_CACHE:
            _NC_CACHE[N] = build(N)
        _RUNNER_CACHE[N] = _make_runner(_NC_CACHE[N])
    return _RUNNER_CACHE[N]


def kernel(x, gn_w, gn_b, w_qkv, b_qkv, w_out, b_out):
    x = np.asarray(x, dtype=np.float32)
    B, _, N = x.shape
    S = N // 2
    run = get_runner(N)
    in_maps = host_inputs(x, gn_w, gn_b, w_qkv, b_qkv, w_out, b_out)
    results = run(in_maps)
    out = np.empty((B, C, N), dtype=np.float32)
    for core in range(N_CORES):
        b, half = divmod(core, 2)
        out[b, :, half * S:(half + 1) * S] = results[core]["out"]
    return out


# revision 17
# speedup vs baseline: 1.1185x; 1.1185x over previous
"""AttentionBlock (GroupNorm -> QKV -> full attention -> out-proj + residual)
for B=4, C=128, N=4096 on 8 Trainium2 NeuronCores.

Sharding: 8 cores = 4 batches x 2 query-slabs of N/2. Every core runs the
same program; the host rolls each core's x so its query slab is always
columns [0, N/2).

Matmuls run in float32r (fp32 data, PE rounds to ~tf32 -> ~1e-4 rel err at
bf16 speed). Scores are computed transposed [j, i] so softmax's exp feeds
the PV matmul with no transposes; row sums come from an all-ones matmul
accumulated alongside PV, and the normalization is one multiply by a
DMA-broadcast reciprocal at the end.
"""

import math
import sys

if "/opt/trn_rl_repo" not in sys.path:
    sys.path.insert(0, "/opt/trn_rl_repo")

import numpy as np

C = 128
G = 8
GS = C // G  # channels per group
EPS = 1e-5
N_CORES = 8


def build(N=4096):
    """Build the per-core Bass program. Returns the compiled Bacc module."""
    import concourse.bacc as bacc
    import concourse.bass as bass
    import concourse.mybir as mybir
    import concourse.tile as tile

    f32 = mybir.dt.float32
    f32r = mybir.dt.float32r
    AF = mybir.ActivationFunctionType
    OP = mybir.AluOpType

    S = N // 2           # query slab width per core
    ICW = min(1024, S)   # i-chunk width (one PV/rowsum accumulation pass)
    NIC = S // ICW       # number of i-chunk passes
    SC = min(512, ICW)   # score chunk width (one QK matmul / exp op)
    NSC = ICW // SC      # score chunks per i-chunk
    NJT = N // 128       # number of j (key) tiles
    BNC = min(512, N)    # bn_stats chunk
    NBN = N // BNC
    PCW = min(512, S)    # projection/epilogue chunk width for slab-sized tensors
    NPC = S // PCW
    SCALE = 1.0 / math.sqrt(C)

    nc = bacc.Bacc("TRN2", target_bir_lowering=False, debug=False)

    x_d = nc.dram_tensor("x", [C, N], f32, kind="ExternalInput").ap()
    w_d = nc.dram_tensor("wcat", [C, 4 * C], f32, kind="ExternalInput").ap()
    m_d = nc.dram_tensor("gmask", [C, C], f32, kind="ExternalInput").ap()
    b_d = nc.dram_tensor("bcat", [C, 5], f32, kind="ExternalInput").ap()
    o_d = nc.dram_tensor("out", [C, S], f32, kind="ExternalOutput").ap()
    # DRAM scratch for the rowsum reshape/broadcast round-trips
    rs1_d = nc.dram_tensor("rs_scr1", [NIC, ICW], f32).ap()
    rs2_d = nc.dram_tensor("rs_scr2", [NIC, ICW], f32).ap()

    with tile.TileContext(nc) as tc:
        with tc.tile_pool(name="consts", bufs=1) as cp, \
             tc.tile_pool(name="big", bufs=1) as bp, \
             tc.tile_pool(name="small", bufs=2) as sp_, \
             tc.tile_pool(name="pP", bufs=4) as pP:

            # ---- loads + constants ----
            xS = bp.tile([C, N], f32, tag="x")
            for dc in range(NBN):
                nc.sync.dma_start(xS[:, dc * BNC:(dc + 1) * BNC],
                                  x_d[:, dc * BNC:(dc + 1) * BNC])
            wS = cp.tile([C, 4 * C], f32, tag="w")
            nc.sync.dma_start(wS[:], w_d[:])
            wR = cp.tile([C, 4 * C], f32r, tag="wr")
            nc.vector.tensor_copy(wR[:], wS[:])
            mS = cp.tile([C, C], f32, tag="gmask")
            nc.sync.dma_start(mS[:], m_d[:])
            bS = cp.tile([C, 5], f32, tag="bcat")
            nc.sync.dma_start(bS[:], b_d[:])
            onesS = cp.tile([C, C], f32, tag="ones")
            nc.vector.memset(onesS[:], 1.0)
            onesR = cp.tile([C, C], f32r, tag="onesr")
            nc.vector.tensor_copy(onesR[:], onesS[:])
            epsT = cp.tile([C, 1], f32, tag="eps")
            nc.vector.memset(epsT[:], EPS)
            bf16 = mybir.dt.bfloat16
            wvB = cp.tile([C, C], bf16, tag="wvB")
            nc.vector.tensor_copy(wvB[:], wS[:, 2 * C:3 * C])

            hR = bp.tile([C, N], f32r, tag="h")
            kR = bp.tile([C, N], f32r, tag="k")
            qR = bp.tile([C, S], f32r, tag="q")
            vTR = bp.tile([C, N], f32r, tag="vT")
            h2nR = bp.tile([C, S], f32r, tag="h2n")
            outS = bp.tile([C, S], f32, tag="outS")

            with tc.tile_pool(name="ps_pre", bufs=2, space="PSUM") as pre:
                # ---- GroupNorm stats ----
                st6 = sp_.tile([C, NBN, 6], f32, tag="st6")
                for i in range(NBN):
                    nc.vector.bn_stats(out=st6[:, i, :], in_=xS[:, i * BNC:(i + 1) * BNC])
                mv = sp_.tile([C, 2], f32, tag="mv")
                nc.vector.bn_aggr(out=mv[:], in_=st6[:])
                # stats2 = [mean, E[x^2]] per channel
                st2 = sp_.tile([C, 2], f32, tag="st2")
                nc.vector.tensor_copy(st2[:, 0:1], mv[:, 0:1])
                nc.vector.tensor_tensor(out=st2[:, 1:2], in0=mv[:, 0:1], in1=mv[:, 0:1], op=OP.mult)
                nc.vector.tensor_tensor(out=st2[:, 1:2], in0=st2[:, 1:2], in1=mv[:, 1:2], op=OP.add)
                # cross-partition group reduce: gstats[c,:] = [gmean, gEx2] of c's group
                gps = pre.tile([C, 2], f32, tag="gstats")
                nc.tensor.matmul(gps[:], mS[:], st2[:], start=True, stop=True)
                gst = sp_.tile([C, 2], f32, tag="gst")
                nc.vector.tensor_copy(gst[:], gps[:])
                gv = sp_.tile([C, 1], f32, tag="gv")
                nc.vector.tensor_tensor(out=gv[:], in0=gst[:, 0:1], in1=gst[:, 0:1], op=OP.mult)
                nc.vector.tensor_tensor(out=gv[:], in0=gst[:, 1:2], in1=gv[:], op=OP.subtract)
                # inv = rsqrt(gv + eps) on DVE: Quake seed + 3 Newton steps
                i32 = mybir.dt.int32
                xv = sp_.tile([C, 1], f32, tag="xv")
                nc.vector.tensor_tensor(out=xv[:], in0=gv[:], in1=epsT[:], op=OP.add)
                magicT = cp.tile([C, 1], i32, tag="magic")
                nc.vector.memset(magicT[:], 0x5F3759DF)
                yh = sp_.tile([C, 1], i32, tag="yh")
                nc.vector.tensor_scalar(out=yh[:], in0=xv[:].bitcast(i32), scalar1=1,
                                        scalar2=None, op0=OP.logical_shift_right)
                nc.vector.tensor_tensor(out=yh[:], in0=magicT[:], in1=yh[:], op=OP.subtract)
                inv = sp_.tile([C, 1], f32, tag="inv")
                nc.vector.tensor_copy(inv[:], yh[:].bitcast(f32))
                tN = sp_.tile([C, 1], f32, tag="tN")
                for _ in range(3):
                    nc.vector.tensor_tensor(out=tN[:], in0=inv[:], in1=inv[:], op=OP.mult)
                    nc.vector.tensor_tensor(out=tN[:], in0=tN[:], in1=xv[:], op=OP.mult)
                    nc.vector.tensor_scalar(out=tN[:], in0=tN[:], scalar1=-0.5,
                                            scalar2=1.5, op0=OP.mult, op1=OP.add)
                    nc.vector.tensor_tensor(out=inv[:], in0=inv[:], in1=tN[:], op=OP.mult)
                aT = sp_.tile([C, 1], f32, tag="aT")
                nc.vector.tensor_tensor(out=aT[:], in0=bS[:, 3:4], in1=inv[:], op=OP.mult)
                bT = sp_.tile([C, 1], f32, tag="bT")
                nc.vector.tensor_tensor(out=bT[:], in0=gst[:, 0:1], in1=aT[:], op=OP.mult)
                nc.vector.tensor_tensor(out=bT[:], in0=bS[:, 4:5], in1=bT[:], op=OP.subtract)
                # h = a*x + b  (rounded to f32r), chunked so projections can
                # start before the whole affine finishes
                for c in range(N // BNC):
                    nc.vector.tensor_scalar(out=hR[:, c * BNC:(c + 1) * BNC],
                                            in0=xS[:, c * BNC:(c + 1) * BNC],
                                            scalar1=aT[:], scalar2=bT[:],
                                            op0=OP.mult, op1=OP.add)

                # ---- projections ----
                hB = bp.tile([C, N], bf16, tag="hB")
                for c in range(N // BNC):
                    nc.vector.tensor_copy(hB[:, c * BNC:(c + 1) * BNC],
                                          hR[:, c * BNC:(c + 1) * BNC].bitcast(f32))
                for c in range(NPC):
                    qp = pre.tile([C, PCW], f32, tag="qp")
                    nc.tensor.matmul(qp[:], wR[:, 0:C], hR[:, PCW * c:PCW * (c + 1)],
                                     start=True, stop=True)
                    nc.scalar.activation(out=qR[:, PCW * c:PCW * (c + 1)], in_=qp[:],
                                         func=AF.Identity, bias=bS[:, 0:1], scale=1.0)
                for c in range(N // 512):
                    kp = pre.tile([C, 512], f32, tag="kp")
                    nc.tensor.matmul(kp[:], wR[:, C:2 * C], hR[:, 512 * c:512 * (c + 1)],
                                     start=True, stop=True)
                    nc.scalar.activation(out=kR[:, 512 * c:512 * (c + 1)], in_=kp[:],
                                         func=AF.Identity, bias=bS[:, 1:2], scale=1.0)
                for g in range(N // 512):
                    vtp = pre.tile([C, 512], f32, tag="vtp")
                    for j4 in range(4):
                        t = 4 * g + j4
                        nc.tensor.matmul(vtp[:, 128 * j4:128 * (j4 + 1)],
                                         hB[:, 128 * t:128 * (t + 1)], wvB[:],
                                         start=True, stop=True)
                    nc.vector.tensor_copy(vTR[:, 512 * g:512 * (g + 1)], vtp[:])

            # ---- attention ----
            # sT double-buffered at STW wide (one exp op per tile). PE runs in
            # program order, so PV/ones for tile t are emitted one tile late:
            # while exp(t) runs on ACT, PE issues QK(t+1) instead of stalling.
            STW = min(1024, ICW)   # sT tile / exp chunk width
            NST = ICW // STW
            MMW = min(512, STW)    # matmul free-dim chunk
            NMM = STW // MMW
            h2cS = bp.tile([C, S], f32, tag="h2c")
            NPAIR = NJT // 2  # rowsum matmuls run on pairwise P-sums (DVE adds)
            with tc.tile_pool(name="ps_sT", bufs=2, space="PSUM") as psT, \
                 tc.tile_pool(name="ps_h2", bufs=1, space="PSUM") as ph2, \
                 tc.tile_pool(name="ps_rs", bufs=1, space="PSUM") as prs:
                acc = {}        # ic -> (h2p, rsp)
                pend_pv = None  # (ic, t, Pt) awaiting PV emission
                prev_pt = None  # Pt of the previous (even-t) tile in this pass
                pend_ones = None  # (ic, pair_idx, Ps2) awaiting ones-MM emission

                def emit_pv(p):
                    icp, tp, Ptp = p
                    h2p = acc[icp][0]
                    for m in range(NMM):
                        nc.tensor.matmul(
                            h2p[:, m * MMW:(m + 1) * MMW],
                            vTR[:, 128 * tp:128 * (tp + 1)],
                            Ptp[:, m * MMW:(m + 1) * MMW],
                            start=(tp == 0), stop=(tp == NJT - 1))

                def emit_ones(p, first=None, last=None):
                    icp, pi, Ps2p = p
                    rsp = acc[icp][1]
                    st = first if first is not None else (pi == 0)
                    sp2 = last if last is not None else False
                    for m in range(NMM):
                        nc.tensor.matmul(
                            rsp[:, m * MMW:(m + 1) * MMW], onesR[:],
                            Ps2p[:, m * MMW:(m + 1) * MMW],
                            start=st, stop=sp2)

                def finish_pass(ic):
                    h2p, rsp = acc[ic]
                    FCW = min(512, ICW)
                    for fc in range(ICW // FCW):
                        sl_i = slice(ic * ICW + fc * FCW, ic * ICW + (fc + 1) * FCW)
                        sl_f = slice(fc * FCW, (fc + 1) * FCW)
                        nc.vector.tensor_copy(h2cS[:, sl_i], h2p[:, sl_f])
                        recipB = sp_.tile([C, FCW], f32, tag="recipB")
                        nc.vector.reciprocal_approx_fast(out=recipB[:], in_=rsp[:, sl_f])
                        nc.vector.tensor_tensor(out=h2nR[:, sl_i], in0=h2cS[:, sl_i],
                                                in1=recipB[:], op=OP.mult)

                for ic in range(NIC):
                    acc[ic] = (ph2.tile([C, ICW], f32, tag="h2u", name=f"h2u{ic}"),
                               prs.tile([C, ICW], f32, tag="rs", name=f"rs{ic}"))
                    for t in range(NJT):
                        for c2 in range(NST):
                            i0 = ic * ICW + c2 * STW
                            sT = psT.tile([C, STW], f32, tag="sT")
                            for m in range(NMM):
                                nc.tensor.matmul(
                                    sT[:, m * MMW:(m + 1) * MMW],
                                    kR[:, 128 * t:128 * (t + 1)],
                                    qR[:, i0 + m * MMW:i0 + (m + 1) * MMW],
                                    start=True, stop=True)
                            Pt = pP.tile([C, STW], f32r, tag="P")
                            nc.scalar.activation(out=Pt[:], in_=sT[:], func=AF.Exp,
                                                 scale=SCALE)
                            if pend_pv is not None:
                                emit_pv(pend_pv)
                                tp = pend_pv[1]
                                if tp == NJT - 2 or tp == NJT - 1:
                                    # tail of the pass: direct ones-MMs so the
                                    # rowsum doesn't wait on a DVE pair-add
                                    if pend_ones is not None:
                                        emit_ones(pend_ones)
                                        pend_ones = None
                                    emit_ones((pend_pv[0], -1, pend_pv[2]),
                                              first=(tp == 0), last=(tp == NJT - 1))
                                elif tp % 2 == 1:
                                    # DVE pair-sum of the two P tiles just used
                                    Ps2 = pP.tile([C, STW], f32r, tag="Ps2")
                                    nc.vector.tensor_tensor(
                                        out=Ps2[:], in0=prev_pt[:].bitcast(f32),
                                        in1=pend_pv[2][:].bitcast(f32), op=OP.add)
                                    if pend_ones is not None:
                                        emit_ones(pend_ones)
                                    pend_ones = (pend_pv[0], tp // 2, Ps2)
                                else:
                                    prev_pt = pend_pv[2]
                                if tp == NJT - 1:
                                    finish_pass(pend_pv[0])
                            pend_pv = (ic, t, Pt)
                emit_pv(pend_pv)
                if pend_ones is not None:
                    emit_ones(pend_ones)
                    pend_ones = None
                emit_ones((pend_pv[0], -1, pend_pv[2]), first=False, last=True)
                finish_pass(pend_pv[0])

            # ---- out projection + bias + residual ----
            with tc.tile_pool(name="ps_ep", bufs=2, space="PSUM") as pep:
                for c in range(NPC):
                    pop = pep.tile([C, PCW], f32, tag="pop")
                    nc.tensor.matmul(pop[:], wR[:, 3 * C:4 * C],
                                     h2nR[:, PCW * c:PCW * (c + 1)], start=True, stop=True)
                    nc.vector.scalar_tensor_tensor(
                        out=outS[:, PCW * c:PCW * (c + 1)], in0=pop[:], scalar=bS[:, 2:3],
                        in1=xS[:, PCW * c:PCW * (c + 1)], op0=OP.add, op1=OP.add)
                    nc.sync.dma_start(o_d[:, PCW * c:PCW * (c + 1)],
                                      outS[:, PCW * c:PCW * (c + 1)])

    nc.compile()
    return nc


def host_inputs(x, gn_w, gn_b, w_qkv, b_qkv, w_out, b_out):
    """Build the 8 per-core input maps from the full problem inputs."""
    x = np.asarray(x, dtype=np.float32)
    B, _, N = x.shape
    S = N // 2
    w_qkv = np.asarray(w_qkv, np.float32)
    w_out = np.asarray(w_out, np.float32)
    b_qkv = np.asarray(b_qkv, np.float32)
    b_out = np.asarray(b_out, np.float32)
    gn_w = np.asarray(gn_w, np.float32)
    gn_b = np.asarray(gn_b, np.float32)

    wcat = np.concatenate(
        [w_qkv[0:C].T, w_qkv[C:2 * C].T, w_qkv[2 * C:3 * C].T, w_out.T],
        axis=1).astype(np.float32)                      # [C, 4C], each [c_in, c_out]
    gidx = np.arange(C) // GS
    gmask = (gidx[:, None] == gidx[None, :]).astype(np.float32) / GS
    b_eff = b_out + w_out @ b_qkv[2 * C:3 * C]
    bcat = np.stack([b_qkv[0:C], b_qkv[C:2 * C], b_eff, gn_w, gn_b], axis=1)
    bcat = np.ascontiguousarray(bcat, np.float32)       # [C, 5]

    in_maps = []
    for core in range(N_CORES):
        b, half = divmod(core, 2)
        xb = np.roll(x[b], -half * S, axis=1)
        in_maps.append({"x": np.ascontiguousarray(xb), "wcat": wcat,
                        "gmask": gmask, "bcat": bcat})
    return in_maps


_NC_CACHE = {}
_RUNNER_CACHE = {}


def _make_runner(nc):
    """Compile-once runner: replicates bass2jax.run_bass_via_pjrt but keeps the
    jitted sharded callable so repeat executions skip recompilation."""
    import jax
    import concourse.mybir as mybir
    from jax.sharding import Mesh, PartitionSpec
    from jax.experimental.shard_map import shard_map
    from concourse.bass2jax import (_bass_exec_p, install_neuronx_cc_hook,
                                    partition_id_tensor)

    install_neuronx_cc_hook()
    partition_name = nc.partition_id_tensor.name if nc.partition_id_tensor else None
    in_names, out_names, out_avals, zero_shapes = [], [], [], []
    for alloc in nc.m.functions[0].allocations:
        if not isinstance(alloc, mybir.MemoryLocationSet):
            continue
        name = alloc.memorylocations[0].name
        if alloc.kind == "ExternalInput":
            if name == partition_name:
                continue
            in_names.append(name)
        elif alloc.kind == "ExternalOutput":
            out_names.append(name)
            shape = tuple(alloc.tensor_shape)
            dtype = mybir.dt.np(alloc.dtype)
            out_avals.append(jax.core.ShapedArray(shape, dtype))
            zero_shapes.append((shape, dtype))
    n_params = len(in_names)
    all_names = in_names + out_names
    if partition_name is not None:
        all_names = all_names + [partition_name]
    donate = tuple(range(n_params, n_params + len(out_names)))

    def _body(*args):
        operands = list(args)
        if partition_name is not None:
            operands.append(partition_id_tensor())
        return tuple(_bass_exec_p.bind(
            *operands, out_avals=tuple(out_avals), in_names=tuple(all_names),
            out_names=tuple(out_names), lowering_input_output_aliases=(),
            sim_require_finite=True, sim_require_nnan=True, nc=nc))

    devices = jax.devices()[:N_CORES]
    mesh = Mesh(np.asarray(devices), ("core",))
    specs = (PartitionSpec("core"),)
    sharded = jax.jit(
        shard_map(_body, mesh=mesh,
                  in_specs=specs * (n_params + len(out_names)),
                  out_specs=specs * len(out_names), check_rep=False),
        donate_argnums=donate, keep_unused=True)

    def run(in_maps):
        concat_in = [np.concatenate([np.asarray(m[nm]) for m in in_maps], axis=0)
                     for nm in in_names]
        concat_zeros = [np.zeros((N_CORES * s[0], *s[1:]), d) for s, d in zero_shapes]
        out_arrs = sharded(*concat_in, *concat_zeros)
        out_arrs = [np.asarray(a) for a in out_arrs]
        return [{nm: out_arrs[i].reshape(N_CORES, *out_avals[i].shape)[c]
                 for i, nm in enumerate(out_names)} for c in range(N_CORES)]

    return run


def get_runner(N=4096):
    if N not in _RUNNER_CACHE:
        if N not in _NC_CACHE:
            _NC_CACHE[N] = build(N)
        _RUNNER_CACHE[N] = _make_runner(_NC_CACHE[N])
    return _RUNNER_CACHE[N]


def kernel(x, gn_w, gn_b, w_qkv, b_qkv, w_out, b_out):
    x = np.asarray(x, dtype=np.float32)
    B, _, N = x.shape
    S = N // 2
    run = get_runner(N)
    in_maps = host_inputs(x, gn_w, gn_b, w_qkv, b_qkv, w_out, b_out)
    results = run(in_maps)
    out = np.empty((B, C, N), dtype=np.float32)
    for core in range(N_CORES):
        b, half = divmod(core, 2)
        out[b, :, half * S:(half + 1) * S] = results[core]["out"]
    return out


# revision 25
# speedup vs baseline: 7227.8728x; 6461.8853x over previous
"""AttentionBlock (GroupNorm -> QKV -> full attention -> out-proj + residual)
for B=4, C=128, N=4096 on 8 Trainium2 NeuronCores.

Sharding: 8 cores = 4 batches x 2 query-slabs of N/2. Every core runs the
same program; the host rolls each core's x so its query slab is always
columns [0, N/2).

Matmuls run in float32r (fp32 data, PE rounds to ~tf32 -> ~1e-4 rel err at
bf16 speed). Scores are computed transposed [j, i] so softmax's exp feeds
the PV matmul with no transposes; row sums come from an all-ones matmul
accumulated alongside PV, and the normalization is one multiply by a
DMA-broadcast reciprocal at the end.
"""

import math
import sys

if "/opt/trn_rl_repo" not in sys.path:
    sys.path.insert(0, "/opt/trn_rl_repo")

import numpy as np

C = 128
G = 8
GS = C // G  # channels per group
EPS = 1e-5
N_CORES = 8


def build(N=4096, repeat=1):
    """Build the per-core Bass program. Returns the compiled Bacc module."""
    import concourse.bacc as bacc
    import concourse.bass as bass
    import concourse.mybir as mybir
    import concourse.tile as tile

    f32 = mybir.dt.float32
    f32r = mybir.dt.float32r
    AF = mybir.ActivationFunctionType
    OP = mybir.AluOpType

    S = N // 2           # query slab width per core
    ICW = min(1024, S)   # i-chunk width (one PV/rowsum accumulation pass)
    NIC = S // ICW       # number of i-chunk passes
    SC = min(512, ICW)   # score chunk width (one QK matmul / exp op)
    NSC = ICW // SC      # score chunks per i-chunk
    NJT = N // 128       # number of j (key) tiles
    BNC = min(512, N)    # bn_stats chunk
    NBN = N // BNC
    PCW = min(512, S)    # projection/epilogue chunk width for slab-sized tensors
    NPC = S // PCW
    SCALE = 1.0 / math.sqrt(C)

    nc = bacc.Bacc("TRN2", target_bir_lowering=False, debug=False)

    x_d = nc.dram_tensor("x", [C, N], f32, kind="ExternalInput").ap()
    w_d = nc.dram_tensor("wcat", [C, 4 * C], f32, kind="ExternalInput").ap()
    m_d = nc.dram_tensor("gmask", [C, C], f32, kind="ExternalInput").ap()
    b_d = nc.dram_tensor("bcat", [C, 5], f32, kind="ExternalInput").ap()
    o_d = nc.dram_tensor("out", [C, S], f32, kind="ExternalOutput").ap()
    # DRAM scratch for the rowsum reshape/broadcast round-trips
    rs1_d = nc.dram_tensor("rs_scr1", [NIC, ICW], f32).ap()
    rs2_d = nc.dram_tensor("rs_scr2", [NIC, ICW], f32).ap()

    with tile.TileContext(nc) as tc:
        with tc.tile_pool(name="consts", bufs=1) as cp, \
             tc.tile_pool(name="big", bufs=1) as bp, \
             tc.tile_pool(name="small", bufs=3) as sp_, \
             tc.tile_pool(name="pP", bufs=6) as pP:
            _loop = tc.For_i(0, repeat, 1) if repeat > 1 else None
            if _loop is not None:
                _loop.__enter__()

            # ---- loads + constants ----
            xS = bp.tile([C, N], f32, tag="x")
            for dc in range(NBN):
                nc.sync.dma_start(xS[:, dc * BNC:(dc + 1) * BNC],
                                  x_d[:, dc * BNC:(dc + 1) * BNC])
            wS = cp.tile([C, 4 * C], f32, tag="w")
            nc.sync.dma_start(wS[:], w_d[:])
            wR = cp.tile([C, 4 * C], f32r, tag="wr")
            nc.vector.tensor_copy(wR[:], wS[:])
            mS = cp.tile([C, C], f32, tag="gmask")
            nc.sync.dma_start(mS[:], m_d[:])
            bS = cp.tile([C, 5], f32, tag="bcat")
            nc.sync.dma_start(bS[:], b_d[:])
            onesS = cp.tile([C, C], f32, tag="ones")
            nc.vector.memset(onesS[:], 1.0)
            onesR = cp.tile([C, C], f32r, tag="onesr")
            nc.vector.tensor_copy(onesR[:], onesS[:])
            epsT = cp.tile([C, 1], f32, tag="eps")
            nc.vector.memset(epsT[:], EPS)
            bf16 = mybir.dt.bfloat16
            wvB = cp.tile([C, C], bf16, tag="wvB")
            nc.vector.tensor_copy(wvB[:], wS[:, 2 * C:3 * C])

            hR = bp.tile([C, N], f32r, tag="h")
            kR = bp.tile([C, N], f32r, tag="k")
            qR = bp.tile([C, S], f32r, tag="q")
            vTR = bp.tile([C, N], f32r, tag="vT")
            h2nR = bp.tile([C, S], f32r, tag="h2n")
            outS = bp.tile([C, S], f32, tag="outS")

            with tc.tile_pool(name="ps_pre", bufs=2, space="PSUM") as pre:
                # ---- GroupNorm stats ----
                st6 = sp_.tile([C, NBN, 6], f32, tag="st6")
                for i in range(NBN):
                    nc.vector.bn_stats(out=st6[:, i, :], in_=xS[:, i * BNC:(i + 1) * BNC])
                mv = sp_.tile([C, 2], f32, tag="mv")
                nc.vector.bn_aggr(out=mv[:], in_=st6[:])
                # stats2 = [mean, E[x^2]] per channel
                st2 = sp_.tile([C, 2], f32, tag="st2")
                nc.vector.tensor_copy(st2[:, 0:1], mv[:, 0:1])
                nc.vector.tensor_tensor(out=st2[:, 1:2], in0=mv[:, 0:1], in1=mv[:, 0:1], op=OP.mult)
                nc.vector.tensor_tensor(out=st2[:, 1:2], in0=st2[:, 1:2], in1=mv[:, 1:2], op=OP.add)
                # cross-partition group reduce: gstats[c,:] = [gmean, gEx2] of c's group
                gps = pre.tile([C, 2], f32, tag="gstats")
                nc.tensor.matmul(gps[:], mS[:], st2[:], start=True, stop=True)
                gst = sp_.tile([C, 2], f32, tag="gst")
                nc.vector.tensor_copy(gst[:], gps[:])
                gv = sp_.tile([C, 1], f32, tag="gv")
                nc.vector.tensor_tensor(out=gv[:], in0=gst[:, 0:1], in1=gst[:, 0:1], op=OP.mult)
                nc.vector.tensor_tensor(out=gv[:], in0=gst[:, 1:2], in1=gv[:], op=OP.subtract)
                # inv = rsqrt(gv + eps) on DVE: Quake seed + 3 Newton steps
                i32 = mybir.dt.int32
                xv = sp_.tile([C, 1], f32, tag="xv")
                nc.vector.tensor_tensor(out=xv[:], in0=gv[:], in1=epsT[:], op=OP.add)
                magicT = cp.tile([C, 1], i32, tag="magic")
                nc.vector.memset(magicT[:], 0x5F3759DF)
                yh = sp_.tile([C, 1], i32, tag="yh")
                nc.vector.tensor_scalar(out=yh[:], in0=xv[:].bitcast(i32), scalar1=1,
                                        scalar2=None, op0=OP.logical_shift_right)
                nc.vector.tensor_tensor(out=yh[:], in0=magicT[:], in1=yh[:], op=OP.subtract)
                inv = sp_.tile([C, 1], f32, tag="inv")
                nc.vector.tensor_copy(inv[:], yh[:].bitcast(f32))
                tN = sp_.tile([C, 1], f32, tag="tN")
                for _ in range(2):
                    nc.vector.tensor_tensor(out=tN[:], in0=inv[:], in1=inv[:], op=OP.mult)
                    nc.vector.tensor_tensor(out=tN[:], in0=tN[:], in1=xv[:], op=OP.mult)
                    nc.vector.tensor_scalar(out=tN[:], in0=tN[:], scalar1=-0.5,
                                            scalar2=1.5, op0=OP.mult, op1=OP.add)
                    nc.vector.tensor_tensor(out=inv[:], in0=inv[:], in1=tN[:], op=OP.mult)
                aT = sp_.tile([C, 1], f32, tag="aT")
                nc.vector.tensor_tensor(out=aT[:], in0=bS[:, 3:4], in1=inv[:], op=OP.mult)
                bT = sp_.tile([C, 1], f32, tag="bT")
                nc.vector.tensor_tensor(out=bT[:], in0=gst[:, 0:1], in1=aT[:], op=OP.mult)
                nc.vector.tensor_tensor(out=bT[:], in0=bS[:, 4:5], in1=bT[:], op=OP.subtract)
                # h = a*x + b  (rounded to f32r), chunked so projections can
                # start before the whole affine finishes
                for c in range(N // BNC):
                    nc.vector.tensor_scalar(out=hR[:, c * BNC:(c + 1) * BNC],
                                            in0=xS[:, c * BNC:(c + 1) * BNC],
                                            scalar1=aT[:], scalar2=bT[:],
                                            op0=OP.mult, op1=OP.add)

                # ---- projections ----
                hB = bp.tile([C, N], bf16, tag="hB")
                for c in range(N // BNC):
                    nc.vector.tensor_copy(hB[:, c * BNC:(c + 1) * BNC],
                                          hR[:, c * BNC:(c + 1) * BNC].bitcast(f32))
                for c in range(NPC):
                    qp = pre.tile([C, PCW], f32, tag="qp")
                    nc.tensor.matmul(qp[:], wR[:, 0:C], hR[:, PCW * c:PCW * (c + 1)],
                                     start=True, stop=True)
                    nc.scalar.activation(out=qR[:, PCW * c:PCW * (c + 1)], in_=qp[:],
                                         func=AF.Identity, bias=bS[:, 0:1], scale=1.0)
                for c in range(N // 512):
                    kp = pre.tile([C, 512], f32, tag="kp")
                    nc.tensor.matmul(kp[:], wR[:, C:2 * C], hR[:, 512 * c:512 * (c + 1)],
                                     start=True, stop=True)
                    nc.scalar.activation(out=kR[:, 512 * c:512 * (c + 1)], in_=kp[:],
                                         func=AF.Identity, bias=bS[:, 1:2], scale=1.0)
                for g in range(N // 512):
                    vtp = pre.tile([C, 512], f32, tag="vtp")
                    for j4 in range(4):
                        tj = 4 * g + j4
                        nc.tensor.matmul(vtp[:, 128 * j4:128 * (j4 + 1)],
                                         hB[:, 128 * tj:128 * (tj + 1)], wvB[:],
                                         start=True, stop=True)
                    nc.vector.tensor_copy(vTR[:, 512 * g:512 * (g + 1)], vtp[:])


            # ---- attention ----
            # sT double-buffered at STW wide (one exp op per tile). PE runs in
            # program order, so PV/ones for tile t are emitted one tile late:
            # while exp(t) runs on ACT, PE issues QK(t+1) instead of stalling.
            STW = min(1024, ICW)   # sT tile / exp chunk width
            NST = ICW // STW
            MMW = min(512, STW)    # matmul free-dim chunk
            NMM = STW // MMW
            NPAIR = NJT // 2  # rowsum matmuls run on pairwise P-sums (DVE adds)
            with tc.tile_pool(name="ps_sT", bufs=2, space="PSUM") as psT, \
                 tc.tile_pool(name="ps_h2", bufs=1, space="PSUM") as ph2, \
                 tc.tile_pool(name="ps_rs", bufs=1, space="PSUM") as prs:
                acc = {}        # ic -> (h2p, rsp)
                pend_pv = None  # (ic, t, Pt) awaiting PV emission
                prev_pt = None  # Pt of the previous (even-t) tile in this pass
                pend_ones = None  # (ic, pair_idx, Ps2) awaiting ones-MM emission

                def emit_pv(p):
                    icp, tp, Ptp = p
                    h2p = acc[icp][0]
                    for m in range(NMM):
                        nc.tensor.matmul(
                            h2p[:, m * MMW:(m + 1) * MMW],
                            vTR[:, 128 * tp:128 * (tp + 1)],
                            Ptp[:, m * MMW:(m + 1) * MMW],
                            start=(tp == 0), stop=(tp == NJT - 1))

                def emit_ones(p, first=None, last=None):
                    icp, pi, Ps2p = p
                    rsp = acc[icp][1]
                    st = first if first is not None else (pi == 0)
                    sp2 = last if last is not None else False
                    for m in range(NMM):
                        nc.tensor.matmul(
                            rsp[:, m * MMW:(m + 1) * MMW], onesR[:],
                            Ps2p[:, m * MMW:(m + 1) * MMW],
                            start=st, stop=sp2)

                def finish_pass(ic):
                    h2p, rsp = acc[ic]
                    FCW = min(512, ICW)
                    for fc in range(ICW // FCW):
                        sl_i = slice(ic * ICW + fc * FCW, ic * ICW + (fc + 1) * FCW)
                        sl_f = slice(fc * FCW, (fc + 1) * FCW)
                        recipB = sp_.tile([C, FCW], f32, tag="recipB")
                        nc.vector.reciprocal_approx_fast(out=recipB[:], in_=rsp[:, sl_f])
                        nc.vector.tensor_tensor(out=h2nR[:, sl_i], in0=h2p[:, sl_f],
                                                in1=recipB[:], op=OP.mult)

                for ic in range(NIC):
                    acc[ic] = (ph2.tile([C, ICW], f32, tag="h2u", name=f"h2u{ic}"),
                               prs.tile([C, ICW], f32, tag="rs", name=f"rs{ic}"))
                    for t in range(NJT):
                        for c2 in range(NST):
                            i0 = ic * ICW + c2 * STW
                            sT = psT.tile([C, STW], f32, tag="sT")
                            for m in range(NMM):
                                nc.tensor.matmul(
                                    sT[:, m * MMW:(m + 1) * MMW],
                                    kR[:, 128 * t:128 * (t + 1)],
                                    qR[:, i0 + m * MMW:i0 + (m + 1) * MMW],
                                    start=True, stop=True)
                            Pt = pP.tile([C, STW], f32r, tag="P")
                            nc.scalar.activation(out=Pt[:], in_=sT[:], func=AF.Exp,
                                                 scale=SCALE)
                            if pend_pv is not None:
                                emit_pv(pend_pv)
                                tp = pend_pv[1]
                                if tp == NJT - 2 or tp == NJT - 1:
                                    # tail of the pass: direct ones-MMs so the
                                    # rowsum doesn't wait on a DVE pair-add
                                    if pend_ones is not None:
                                        emit_ones(pend_ones)
                                        pend_ones = None
                                    emit_ones((pend_pv[0], -1, pend_pv[2]),
                                              first=(tp == 0), last=(tp == NJT - 1))
                                elif tp % 2 == 1:
                                    # DVE pair-sum of the two P tiles just used
                                    Ps2 = pP.tile([C, STW], f32r, tag="Ps2")
                                    nc.vector.tensor_tensor(
                                        out=Ps2[:], in0=prev_pt[:].bitcast(f32),
                                        in1=pend_pv[2][:].bitcast(f32), op=OP.add)
                                    if pend_ones is not None:
                                        emit_ones(pend_ones)
                                    pend_ones = (pend_pv[0], tp // 2, Ps2)
                                else:
                                    prev_pt = pend_pv[2]
                                if tp == NJT - 1:
                                    finish_pass(pend_pv[0])
                            pend_pv = (ic, t, Pt)
                emit_pv(pend_pv)
                if pend_ones is not None:
                    emit_ones(pend_ones)
                    pend_ones = None
                emit_ones((pend_pv[0], -1, pend_pv[2]), first=False, last=True)
                finish_pass(pend_pv[0])

            # ---- out projection + bias + residual ----
            with tc.tile_pool(name="ps_ep", bufs=2, space="PSUM") as pep:
                for c in range(NPC):
                    pop = pep.tile([C, PCW], f32, tag="pop")
                    nc.tensor.matmul(pop[:], wR[:, 3 * C:4 * C],
                                     h2nR[:, PCW * c:PCW * (c + 1)], start=True, stop=True)
                    nc.vector.scalar_tensor_tensor(
                        out=outS[:, PCW * c:PCW * (c + 1)], in0=pop[:], scalar=bS[:, 2:3],
                        in1=xS[:, PCW * c:PCW * (c + 1)], op0=OP.add, op1=OP.add)
                    nc.sync.dma_start(o_d[:, PCW * c:PCW * (c + 1)],
                                      outS[:, PCW * c:PCW * (c + 1)])
            if _loop is not None:
                _loop.__exit__(None, None, None)

    nc.compile()
    return nc


def host_inputs(x, gn_w, gn_b, w_qkv, b_qkv, w_out, b_out):
    """Build the 8 per-core input maps from the full problem inputs."""
    x = np.asarray(x, dtype=np.float32)
    B, _, N = x.shape
    S = N // 2
    w_qkv = np.asarray(w_qkv, np.float32)
    w_out = np.asarray(w_out, np.float32)
    b_qkv = np.asarray(b_qkv, np.float32)
    b_out = np.asarray(b_out, np.float32)
    gn_w = np.asarray(gn_w, np.float32)
    gn_b = np.asarray(gn_b, np.float32)

    wcat = np.concatenate(
        [w_qkv[0:C].T, w_qkv[C:2 * C].T, w_qkv[2 * C:3 * C].T, w_out.T],
        axis=1).astype(np.float32)                      # [C, 4C], each [c_in, c_out]
    gidx = np.arange(C) // GS
    gmask = (gidx[:, None] == gidx[None, :]).astype(np.float32) / GS
    b_eff = b_out + w_out @ b_qkv[2 * C:3 * C]
    bcat = np.stack([b_qkv[0:C], b_qkv[C:2 * C], b_eff, gn_w, gn_b], axis=1)
    bcat = np.ascontiguousarray(bcat, np.float32)       # [C, 5]

    in_maps = []
    for core in range(N_CORES):
        b, half = divmod(core, 2)
        xb = np.roll(x[b], -half * S, axis=1)
        in_maps.append({"x": np.ascontiguousarray(xb), "wcat": wcat,
                        "gmask": gmask, "bcat": bcat})
    return in_maps


_NC_CACHE = {}
_RUNNER_CACHE = {}


def _make_runner(nc):
    """Compile-once runner: replicates bass2jax.run_bass_via_pjrt but keeps the
    jitted sharded callable so repeat executions skip recompilation."""
    import jax
    import concourse.mybir as mybir
    from jax.sharding import Mesh, PartitionSpec
    from jax.experimental.shard_map import shard_map
    from concourse.bass2jax import (_bass_exec_p, install_neuronx_cc_hook,
                                    partition_id_tensor)

    install_neuronx_cc_hook()
    partition_name = nc.partition_id_tensor.name if nc.partition_id_tensor else None
    in_names, out_names, out_avals, zero_shapes = [], [], [], []
    for alloc in nc.m.functions[0].allocations:
        if not isinstance(alloc, mybir.MemoryLocationSet):
            continue
        name = alloc.memorylocations[0].name
        if alloc.kind == "ExternalInput":
            if name == partition_name:
                continue
            in_names.append(name)
        elif alloc.kind == "ExternalOutput":
            out_names.append(name)
            shape = tuple(alloc.tensor_shape)
            dtype = mybir.dt.np(alloc.dtype)
            out_avals.append(jax.core.ShapedArray(shape, dtype))
            zero_shapes.append((shape, dtype))
    n_params = len(in_names)
    all_names = in_names + out_names
    if partition_name is not None:
        all_names = all_names + [partition_name]
    donate = tuple(range(n_params, n_params + len(out_names)))

    def _body(*args):
        operands = list(args)
        if partition_name is not None:
            operands.append(partition_id_tensor())
        return tuple(_bass_exec_p.bind(
            *operands, out_avals=tuple(out_avals), in_names=tuple(all_names),
            out_names=tuple(out_names), lowering_input_output_aliases=(),
            sim_require_finite=True, sim_require_nnan=True, nc=nc))

    devices = jax.devices()[:N_CORES]
    mesh = Mesh(np.asarray(devices), ("core",))
    specs = (PartitionSpec("core"),)
    sharded = jax.jit(
        shard_map(_body, mesh=mesh,
                  in_specs=specs * (n_params + len(out_names)),
                  out_specs=specs * len(out_names), check_rep=False),
        donate_argnums=donate, keep_unused=True)

    def run(in_maps):
        concat_in = [np.concatenate([np.asarray(m[nm]) for m in in_maps], axis=0)
                     for nm in in_names]
        concat_zeros = [np.zeros((N_CORES * s[0], *s[1:]), d) for s, d in zero_shapes]
        out_arrs = sharded(*concat_in, *concat_zeros)
        out_arrs = [np.asarray(a) for a in out_arrs]
        return [{nm: out_arrs[i].reshape(N_CORES, *out_avals[i].shape)[c]
                 for i, nm in enumerate(out_names)} for c in range(N_CORES)]

    return run


def get_runner(N=4096):
    if N not in _RUNNER_CACHE:
        if N not in _NC_CACHE:
            _NC_CACHE[N] = build(N)
        _RUNNER_CACHE[N] = _make_runner(_NC_CACHE[N])
    return _RUNNER_CACHE[N]


def kernel(x, gn_w, gn_b, w_qkv, b_qkv, w_out, b_out):
    x = np.asarray(x, dtype=np.float32)
    B, _, N = x.shape
    S = N // 2
    run = get_runner(N)
    in_maps = host_inputs(x, gn_w, gn_b, w_qkv, b_qkv, w_out, b_out)
    results = run(in_maps)
    out = np.empty((B, C, N), dtype=np.float32)
    for core in range(N_CORES):
        b, half = divmod(core, 2)
        out[b, :, half * S:(half + 1) * S] = results[core]["out"]
    return out


# revision 31
# speedup vs baseline: 7234.9327x; 1.0010x over previous
"""AttentionBlock (GroupNorm -> QKV -> full attention -> out-proj + residual)
for B=4, C=128, N=4096 on 8 Trainium2 NeuronCores.

Sharding: 8 cores = 4 batches x 2 query-slabs of N/2. Every core runs the
same program; the host rolls each core's x so its query slab is always
columns [0, N/2).

Matmuls run in float32r (fp32 data, the PE rounds to ~tf32 -> ~1e-4 rel err
at full speed); the v-transpose projection runs in bf16. Scores are computed
transposed [j, i] so softmax's exp feeds the PV matmul with no transposes.
Row sums come from all-ones matmuls over DVE-pairsummed P tiles, accumulated
in PSUM alongside PV with every partition holding the same row-sum; the
softmax normalization is then one reciprocal_approx_fast + multiply per
pass. PE executes in program order, so PV/ones for tile t are emitted one
tile late (software pipelining against the ACT exp).
"""

import math
import sys

if "/opt/trn_rl_repo" not in sys.path:
    sys.path.insert(0, "/opt/trn_rl_repo")

import numpy as np

C = 128
G = 8
GS = C // G  # channels per group
EPS = 1e-5
N_CORES = 8


def build(N=4096, repeat=1):
    """Build the per-core Bass program. Returns the compiled Bacc module."""
    import concourse.bacc as bacc
    import concourse.bass as bass
    import concourse.mybir as mybir
    import concourse.tile as tile

    f32 = mybir.dt.float32
    f32r = mybir.dt.float32r
    AF = mybir.ActivationFunctionType
    OP = mybir.AluOpType

    S = N // 2           # query slab width per core
    ICW = min(1024, S)   # i-chunk width (one PV/rowsum accumulation pass)
    NIC = S // ICW       # number of i-chunk passes
    NJT = N // 128       # number of j (key) tiles
    BNC = min(512, N)    # bn_stats chunk
    NBN = N // BNC
    PCW = min(512, S)    # projection/epilogue chunk width for slab-sized tensors
    NPC = S // PCW
    SCALE = 1.0 / math.sqrt(C)

    nc = bacc.Bacc("TRN2", target_bir_lowering=False, debug=False)

    x_d = nc.dram_tensor("x", [C, N], f32, kind="ExternalInput").ap()
    w_d = nc.dram_tensor("wcat", [C, 4 * C], f32, kind="ExternalInput").ap()
    m_d = nc.dram_tensor("gmask", [C, C], f32, kind="ExternalInput").ap()
    b_d = nc.dram_tensor("bcat", [C, 5], f32, kind="ExternalInput").ap()
    o_d = nc.dram_tensor("out", [C, S], f32, kind="ExternalOutput").ap()

    with tile.TileContext(nc) as tc:
        with tc.tile_pool(name="consts", bufs=1) as cp, \
             tc.tile_pool(name="big", bufs=1) as bp, \
             tc.tile_pool(name="small", bufs=3) as sp_, \
             tc.tile_pool(name="pP", bufs=6) as pP:
            _loop = tc.For_i(0, repeat, 1) if repeat > 1 else None
            if _loop is not None:
                _loop.__enter__()

            # ---- loads + constants ----
            xS = bp.tile([C, N], f32, tag="x")
            for dc in range(NBN):
                nc.sync.dma_start(xS[:, dc * BNC:(dc + 1) * BNC],
                                  x_d[:, dc * BNC:(dc + 1) * BNC])
            wS = cp.tile([C, 4 * C], f32, tag="w")
            nc.sync.dma_start(wS[:], w_d[:])
            wR = cp.tile([C, 4 * C], f32r, tag="wr")
            nc.vector.tensor_copy(wR[:], wS[:])
            mS = cp.tile([C, C], f32, tag="gmask")
            nc.sync.dma_start(mS[:], m_d[:])
            bS = cp.tile([C, 5], f32, tag="bcat")
            nc.sync.dma_start(bS[:], b_d[:])
            onesS = cp.tile([C, C], f32, tag="ones")
            nc.vector.memset(onesS[:], 1.0)
            onesR = cp.tile([C, C], f32r, tag="onesr")
            nc.vector.tensor_copy(onesR[:], onesS[:])
            epsT = cp.tile([C, 1], f32, tag="eps")
            nc.vector.memset(epsT[:], EPS)
            bf16 = mybir.dt.bfloat16
            wvB = cp.tile([C, C], bf16, tag="wvB")
            nc.vector.tensor_copy(wvB[:], wS[:, 2 * C:3 * C])

            hR = bp.tile([C, N], f32r, tag="h")
            kR = bp.tile([C, N], f32r, tag="k")
            qR = bp.tile([C, S], f32r, tag="q")
            vTR = bp.tile([C, N], f32r, tag="vT")
            h2nR = bp.tile([C, S], f32r, tag="h2n")
            outS = bp.tile([C, S], f32, tag="outS")

            with tc.tile_pool(name="ps_pre", bufs=2, space="PSUM") as pre:
                # ---- GroupNorm stats ----
                st6 = sp_.tile([C, NBN, 6], f32, tag="st6")
                for i in range(NBN):
                    nc.vector.bn_stats(out=st6[:, i, :], in_=xS[:, i * BNC:(i + 1) * BNC])
                mv = sp_.tile([C, 2], f32, tag="mv")
                nc.vector.bn_aggr(out=mv[:], in_=st6[:])
                # mv col1 <- mean^2 + var = E[x^2] (in place)
                nc.vector.scalar_tensor_tensor(out=mv[:, 1:2], in0=mv[:, 0:1],
                                               scalar=mv[:, 0:1], in1=mv[:, 1:2],
                                               op0=OP.mult, op1=OP.add)
                # cross-partition group reduce: gstats[c,:] = [gmean, gEx2] of c's group
                gps = pre.tile([C, 2], f32, tag="gstats")
                nc.tensor.matmul(gps[:], mS[:], mv[:], start=True, stop=True)
                gst = sp_.tile([C, 2], f32, tag="gst")
                nc.vector.tensor_copy(gst[:], gps[:])
                # xv = eps + gEx2 - gmean^2  (group variance + eps)
                i32 = mybir.dt.int32
                gv = sp_.tile([C, 1], f32, tag="gv")
                nc.vector.scalar_tensor_tensor(out=gv[:], in0=gst[:, 0:1],
                                               scalar=gst[:, 0:1], in1=gst[:, 1:2],
                                               op0=OP.mult, op1=OP.subtract)
                xv = sp_.tile([C, 1], f32, tag="xv")
                nc.vector.tensor_tensor(out=xv[:], in0=epsT[:], in1=gv[:], op=OP.subtract)
                magicT = cp.tile([C, 1], i32, tag="magic")
                nc.vector.memset(magicT[:], 0x5F3759DF)
                yh = sp_.tile([C, 1], i32, tag="yh")
                nc.vector.tensor_scalar(out=yh[:], in0=xv[:].bitcast(i32), scalar1=1,
                                        scalar2=None, op0=OP.logical_shift_right)
                nc.vector.tensor_tensor(out=yh[:], in0=magicT[:], in1=yh[:], op=OP.subtract)
                inv = sp_.tile([C, 1], f32, tag="inv")
                nc.vector.tensor_copy(inv[:], yh[:].bitcast(f32))
                tN = sp_.tile([C, 1], f32, tag="tN")
                for _ in range(2):
                    nc.vector.tensor_tensor(out=tN[:], in0=inv[:], in1=inv[:], op=OP.mult)
                    nc.vector.tensor_tensor(out=tN[:], in0=tN[:], in1=xv[:], op=OP.mult)
                    nc.vector.tensor_scalar(out=tN[:], in0=tN[:], scalar1=-0.5,
                                            scalar2=1.5, op0=OP.mult, op1=OP.add)
                    nc.vector.tensor_tensor(out=inv[:], in0=inv[:], in1=tN[:], op=OP.mult)
                aT = sp_.tile([C, 1], f32, tag="aT")
                nc.vector.tensor_tensor(out=aT[:], in0=bS[:, 3:4], in1=inv[:], op=OP.mult)
                bT = sp_.tile([C, 1], f32, tag="bT")
                nc.vector.tensor_tensor(out=bT[:], in0=gst[:, 0:1], in1=aT[:], op=OP.mult)
                nc.vector.tensor_tensor(out=bT[:], in0=bS[:, 4:5], in1=bT[:], op=OP.subtract)
                # h = a*x + b  (rounded to f32r), chunked so projections can
                # start before the whole affine finishes
                for c in range(N // BNC):
                    nc.vector.tensor_scalar(out=hR[:, c * BNC:(c + 1) * BNC],
                                            in0=xS[:, c * BNC:(c + 1) * BNC],
                                            scalar1=aT[:], scalar2=bT[:],
                                            op0=OP.mult, op1=OP.add)

                # ---- projections ----
                hB = bp.tile([C, N], bf16, tag="hB")
                for c in range(N // BNC):
                    nc.vector.tensor_copy(hB[:, c * BNC:(c + 1) * BNC],
                                          hR[:, c * BNC:(c + 1) * BNC].bitcast(f32))
                for c in range(NPC):
                    qp = pre.tile([C, PCW], f32, tag="qp")
                    nc.tensor.matmul(qp[:], wR[:, 0:C], hR[:, PCW * c:PCW * (c + 1)],
                                     start=True, stop=True)
                    nc.scalar.activation(out=qR[:, PCW * c:PCW * (c + 1)], in_=qp[:],
                                         func=AF.Identity, bias=bS[:, 0:1], scale=1.0)
                for c in range(N // 512):
                    kp = pre.tile([C, 512], f32, tag="kp")
                    nc.tensor.matmul(kp[:], wR[:, C:2 * C], hR[:, 512 * c:512 * (c + 1)],
                                     start=True, stop=True)
                    nc.scalar.activation(out=kR[:, 512 * c:512 * (c + 1)], in_=kp[:],
                                         func=AF.Identity, bias=bS[:, 1:2], scale=1.0)
                for g in range(N // 512):
                    vtp = pre.tile([C, 512], f32, tag="vtp")
                    for j4 in range(4):
                        tj = 4 * g + j4
                        nc.tensor.matmul(vtp[:, 128 * j4:128 * (j4 + 1)],
                                         hB[:, 128 * tj:128 * (tj + 1)], wvB[:],
                                         start=True, stop=True)
                    nc.vector.tensor_copy(vTR[:, 512 * g:512 * (g + 1)], vtp[:])


            # ---- attention ----
            # sT double-buffered at STW wide (one exp op per tile). PE runs in
            # program order, so PV/ones for tile t are emitted one tile late:
            # while exp(t) runs on ACT, PE issues QK(t+1) instead of stalling.
            STW = min(1024, ICW)   # sT tile / exp chunk width
            NST = ICW // STW
            MMW = min(512, STW)    # matmul free-dim chunk
            NMM = STW // MMW
            NPAIR = NJT // 2  # rowsum matmuls run on pairwise P-sums (DVE adds)
            with tc.tile_pool(name="ps_sT", bufs=2, space="PSUM") as psT, \
                 tc.tile_pool(name="ps_h2", bufs=1, space="PSUM") as ph2, \
                 tc.tile_pool(name="ps_rs", bufs=1, space="PSUM") as prs:
                acc = {}        # ic -> (h2p, rsp)
                pend_pv = None  # (ic, t, Pt) awaiting PV emission
                prev_pt = None  # Pt of the previous (even-t) tile in this pass
                pend_ones = None  # (ic, pair_idx, Ps2) awaiting ones-MM emission

                def emit_pv(p):
                    icp, tp, Ptp = p
                    h2p = acc[icp][0]
                    for m in range(NMM):
                        nc.tensor.matmul(
                            h2p[:, m * MMW:(m + 1) * MMW],
                            vTR[:, 128 * tp:128 * (tp + 1)],
                            Ptp[:, m * MMW:(m + 1) * MMW],
                            start=(tp == 0), stop=(tp == NJT - 1))

                def emit_ones(p, first=None, last=None):
                    icp, pi, Ps2p = p
                    rsp = acc[icp][1]
                    st = first if first is not None else (pi == 0)
                    sp2 = last if last is not None else False
                    for m in range(NMM):
                        nc.tensor.matmul(
                            rsp[:, m * MMW:(m + 1) * MMW], onesR[:],
                            Ps2p[:, m * MMW:(m + 1) * MMW],
                            start=st, stop=sp2)

                def finish_pass(ic):
                    h2p, rsp = acc[ic]
                    FCW = min(512, ICW)
                    for fc in range(ICW // FCW):
                        sl_i = slice(ic * ICW + fc * FCW, ic * ICW + (fc + 1) * FCW)
                        sl_f = slice(fc * FCW, (fc + 1) * FCW)
                        recipB = sp_.tile([C, FCW], f32, tag="recipB")
                        nc.vector.reciprocal_approx_fast(out=recipB[:], in_=rsp[:, sl_f])
                        nc.vector.tensor_tensor(out=h2nR[:, sl_i], in0=h2p[:, sl_f],
                                                in1=recipB[:], op=OP.mult)

                for ic in range(NIC):
                    acc[ic] = (ph2.tile([C, ICW], f32, tag="h2u", name=f"h2u{ic}"),
                               prs.tile([C, ICW], f32, tag="rs", name=f"rs{ic}"))
                    for t in range(NJT):
                        for c2 in range(NST):
                            i0 = ic * ICW + c2 * STW
                            sT = psT.tile([C, STW], f32, tag="sT")
                            for m in range(NMM):
                                nc.tensor.matmul(
                                    sT[:, m * MMW:(m + 1) * MMW],
                                    kR[:, 128 * t:128 * (t + 1)],
                                    qR[:, i0 + m * MMW:i0 + (m + 1) * MMW],
                                    start=True, stop=True)
                            Pt = pP.tile([C, STW], f32r, tag="P")
                            nc.scalar.activation(out=Pt[:], in_=sT[:], func=AF.Exp,
                                                 scale=SCALE)
                            if pend_pv is not None:
                                emit_pv(pend_pv)
                                tp = pend_pv[1]
                                if tp == NJT - 2 or tp == NJT - 1:
                                    # tail of the pass: direct ones-MMs so the
                                    # rowsum doesn't wait on a DVE pair-add
                                    if pend_ones is not None:
                                        emit_ones(pend_ones)
                                        pend_ones = None
                                    emit_ones((pend_pv[0], -1, pend_pv[2]),
                                              first=(tp == 0), last=(tp == NJT - 1))
                                elif tp % 2 == 1:
                                    # DVE pair-sum of the two P tiles just used
                                    Ps2 = pP.tile([C, STW], f32r, tag="Ps2")
                                    nc.vector.tensor_tensor(
                                        out=Ps2[:], in0=prev_pt[:].bitcast(f32),
                                        in1=pend_pv[2][:].bitcast(f32), op=OP.add)
                                    if pend_ones is not None:
                                        emit_ones(pend_ones)
                                    pend_ones = (pend_pv[0], tp // 2, Ps2)
                                else:
                                    prev_pt = pend_pv[2]
                                if tp == NJT - 1:
                                    finish_pass(pend_pv[0])
                            pend_pv = (ic, t, Pt)
                emit_pv(pend_pv)
                if pend_ones is not None:
                    emit_ones(pend_ones)
                    pend_ones = None
                emit_ones((pend_pv[0], -1, pend_pv[2]), first=False, last=True)
                finish_pass(pend_pv[0])

            # ---- out projection + bias + residual ----
            with tc.tile_pool(name="ps_ep", bufs=2, space="PSUM") as pep:
                for c in range(NPC):
                    pop = pep.tile([C, PCW], f32, tag="pop")
                    nc.tensor.matmul(pop[:], wR[:, 3 * C:4 * C],
                                     h2nR[:, PCW * c:PCW * (c + 1)], start=True, stop=True)
                    nc.vector.scalar_tensor_tensor(
                        out=outS[:, PCW * c:PCW * (c + 1)], in0=pop[:], scalar=bS[:, 2:3],
                        in1=xS[:, PCW * c:PCW * (c + 1)], op0=OP.add, op1=OP.add)
                    nc.sync.dma_start(o_d[:, PCW * c:PCW * (c + 1)],
                                      outS[:, PCW * c:PCW * (c + 1)])
            if _loop is not None:
                _loop.__exit__(None, None, None)

    nc.compile()
    return nc


def host_inputs(x, gn_w, gn_b, w_qkv, b_qkv, w_out, b_out):
    """Build the 8 per-core input maps from the full problem inputs."""
    x = np.asarray(x, dtype=np.float32)
    B, _, N = x.shape
    S = N // 2
    w_qkv = np.asarray(w_qkv, np.float32)
    w_out = np.asarray(w_out, np.float32)
    b_qkv = np.asarray(b_qkv, np.float32)
    b_out = np.asarray(b_out, np.float32)
    gn_w = np.asarray(gn_w, np.float32)
    gn_b = np.asarray(gn_b, np.float32)

    wcat = np.concatenate(
        [w_qkv[0:C].T, w_qkv[C:2 * C].T, w_qkv[2 * C:3 * C].T, w_out.T],
        axis=1).astype(np.float32)                      # [C, 4C], each [c_in, c_out]
    gidx = np.arange(C) // GS
    gmask = (gidx[:, None] == gidx[None, :]).astype(np.float32) / GS
    b_eff = b_out + w_out @ b_qkv[2 * C:3 * C]
    bcat = np.stack([b_qkv[0:C], b_qkv[C:2 * C], b_eff, gn_w, gn_b], axis=1)
    bcat = np.ascontiguousarray(bcat, np.float32)       # [C, 5]

    in_maps = []
    for core in range(N_CORES):
        b, half = divmod(core, 2)
        xb = np.roll(x[b], -half * S, axis=1)
        in_maps.append({"x": np.ascontiguousarray(xb), "wcat": wcat,
                        "gmask": gmask, "bcat": bcat})
    return in_maps


_NC_CACHE = {}
_RUNNER_CACHE = {}


def _make_runner(nc):
    """Compile-once runner: replicates bass2jax.run_bass_via_pjrt but keeps the
    jitted sharded callable so repeat executions skip recompilation."""
    import jax
    import concourse.mybir as mybir
    from jax.sharding import Mesh, PartitionSpec
    from jax.experimental.shard_map import shard_map
    from concourse.bass2jax import (_bass_exec_p, install_neuronx_cc_hook,
                                    partition_id_tensor)

    install_neuronx_cc_hook()
    partition_name = nc.partition_id_tensor.name if nc.partition_id_tensor else None
    in_names, out_names, out_avals, zero_shapes = [], [], [], []
    for alloc in nc.m.functions[0].allocations:
        if not isinstance(alloc, mybir.MemoryLocationSet):
            continue
        name = alloc.memorylocations[0].name
        if alloc.kind == "ExternalInput":
            if name == partition_name:
                continue
            in_names.append(name)
        elif alloc.kind == "ExternalOutput":
            out_names.append(name)
            shape = tuple(alloc.tensor_shape)
            dtype = mybir.dt.np(alloc.dtype)
            out_avals.append(jax.core.ShapedArray(shape, dtype))
            zero_shapes.append((shape, dtype))
    n_params = len(in_names)
    all_names = in_names + out_names
    if partition_name is not None:
        all_names = all_names + [partition_name]
    donate = tuple(range(n_params, n_params + len(out_names)))

    def _body(*args):
        operands = list(args)
        if partition_name is not None:
            operands.append(partition_id_tensor())
        return tuple(_bass_exec_p.bind(
            *operands, out_avals=tuple(out_avals), in_names=tuple(all_names),
            out_names=tuple(out_names), lowering_input_output_aliases=(),
            sim_require_finite=True, sim_require_nnan=True, nc=nc))

    devices = jax.devices()[:N_CORES]
    mesh = Mesh(np.asarray(devices), ("core",))
    specs = (PartitionSpec("core"),)
    sharded = jax.jit(
        shard_map(_body, mesh=mesh,
                  in_specs=specs * (n_params + len(out_names)),
                  out_specs=specs * len(out_names), check_rep=False),
        donate_argnums=donate, keep_unused=True)

    def run(in_maps):
        concat_in = [np.concatenate([np.asarray(m[nm]) for m in in_maps], axis=0)
                     for nm in in_names]
        concat_zeros = [np.zeros((N_CORES * s[0], *s[1:]), d) for s, d in zero_shapes]
        out_arrs = sharded(*concat_in, *concat_zeros)
        out_arrs = [np.asarray(a) for a in out_arrs]
        return [{nm: out_arrs[i].reshape(N_CORES, *out_avals[i].shape)[c]
                 for i, nm in enumerate(out_names)} for c in range(N_CORES)]

    return run


def get_runner(N=4096):
    if N not in _RUNNER_CACHE:
        if N not in _NC_CACHE:
            _NC_CACHE[N] = build(N)
        _RUNNER_CACHE[N] = _make_runner(_NC_CACHE[N])
    return _RUNNER_CACHE[N]


def kernel(x, gn_w, gn_b, w_qkv, b_qkv, w_out, b_out):
    x = np.asarray(x, dtype=np.float32)
    B, _, N = x.shape
    S = N // 2
    run = get_runner(N)
    in_maps = host_inputs(x, gn_w, gn_b, w_qkv, b_qkv, w_out, b_out)
    results = run(in_maps)
    out = np.empty((B, C, N), dtype=np.float32)
    for core in range(N_CORES):
        b, half = divmod(core, 2)
        out[b, :, half * S:(half + 1) * S] = results[core]["out"]
    return out


# revision 32
# speedup vs baseline: 7742.2632x; 1.0701x over previous
"""AttentionBlock (GroupNorm -> QKV -> full attention -> out-proj + residual)
for B=4, C=128, N=4096 on 8 Trainium2 NeuronCores.

Sharding: 8 cores = 4 batches x 2 query-slabs of N/2. Every core runs the
same program; the host rolls each core's x so its query slab is always
columns [0, N/2).

Matmuls run in float32r (fp32 data, the PE rounds to ~tf32 -> ~1e-4 rel err
at full speed); the v-transpose projection runs in bf16. Scores are computed
transposed [j, i] so softmax's exp feeds the PV matmul with no transposes.
Row sums come from all-ones matmuls over DVE-pairsummed P tiles, accumulated
in PSUM alongside PV with every partition holding the same row-sum; the
softmax normalization is then one reciprocal_approx_fast + multiply per
pass. PE executes in program order, so PV/ones for tile t are emitted one
tile late (software pipelining against the ACT exp).
"""

import math
import sys

if "/opt/trn_rl_repo" not in sys.path:
    sys.path.insert(0, "/opt/trn_rl_repo")

import numpy as np

C = 128
G = 8
GS = C // G  # channels per group
EPS = 1e-5
N_CORES = 8


def build(N=4096, repeat=1):
    """Build the per-core Bass program. Returns the compiled Bacc module."""
    import concourse.bacc as bacc
    import concourse.bass as bass
    import concourse.mybir as mybir
    import concourse.tile as tile

    f32 = mybir.dt.float32
    f32r = mybir.dt.float32r
    AF = mybir.ActivationFunctionType
    OP = mybir.AluOpType

    S = N // 2           # query slab width per core
    ICW = min(1024, S)   # i-chunk width (one PV/rowsum accumulation pass)
    NIC = S // ICW       # number of i-chunk passes
    NJT = N // 128       # number of j (key) tiles
    BNC = min(512, N)    # bn_stats chunk
    NBN = N // BNC
    PCW = min(512, S)    # projection/epilogue chunk width for slab-sized tensors
    NPC = S // PCW
    SCALE = 1.0 / math.sqrt(C)

    nc = bacc.Bacc("TRN2", target_bir_lowering=False, debug=False)

    x_d = nc.dram_tensor("x", [C, N], f32, kind="ExternalInput").ap()
    w_d = nc.dram_tensor("wcat", [C, 4 * C], f32, kind="ExternalInput").ap()
    m_d = nc.dram_tensor("gmask", [C, C], f32, kind="ExternalInput").ap()
    b_d = nc.dram_tensor("bcat", [C, 5], f32, kind="ExternalInput").ap()
    o_d = nc.dram_tensor("out", [C, S], f32, kind="ExternalOutput").ap()

    with tile.TileContext(nc) as tc:
        with tc.tile_pool(name="consts", bufs=1) as cp, \
             tc.tile_pool(name="big", bufs=1) as bp, \
             tc.tile_pool(name="small", bufs=3) as sp_, \
             tc.tile_pool(name="pP", bufs=6) as pP:
            _loop = tc.For_i(0, repeat, 1) if repeat > 1 else None
            if _loop is not None:
                _loop.__enter__()

            # ---- loads + constants ----
            xS = bp.tile([C, N], f32, tag="x")
            for dc in range(NBN):
                nc.sync.dma_start(xS[:, dc * BNC:(dc + 1) * BNC],
                                  x_d[:, dc * BNC:(dc + 1) * BNC])
            wS = cp.tile([C, 4 * C], f32, tag="w")
            nc.sync.dma_start(wS[:], w_d[:])
            wR = cp.tile([C, 4 * C], f32r, tag="wr")
            nc.vector.tensor_copy(wR[:], wS[:])
            mS = cp.tile([C, C], f32, tag="gmask")
            nc.sync.dma_start(mS[:], m_d[:])
            bS = cp.tile([C, 5], f32, tag="bcat")
            nc.sync.dma_start(bS[:], b_d[:])
            onesS = cp.tile([C, C], f32, tag="ones")
            nc.vector.memset(onesS[:], 1.0)
            onesR = cp.tile([C, C], f32r, tag="onesr")
            nc.vector.tensor_copy(onesR[:], onesS[:])
            epsT = cp.tile([C, 1], f32, tag="eps")
            nc.vector.memset(epsT[:], EPS)
            bf16 = mybir.dt.bfloat16
            f8 = mybir.dt.float8e4
            wvB = cp.tile([C, C], bf16, tag="wvB")
            nc.vector.tensor_copy(wvB[:], wS[:, 2 * C:3 * C])
            onesF8 = cp.tile([C, C], f8, tag="onesf8")
            nc.vector.tensor_copy(onesF8[:], onesS[:])

            hR = bp.tile([C, N], f32r, tag="h")
            kR = bp.tile([C, N], f32r, tag="k")
            qR = bp.tile([C, S], f32r, tag="q")
            vTR = bp.tile([C, N], f8, tag="vT")
            h2nR = bp.tile([C, S], f32r, tag="h2n")
            outS = bp.tile([C, S], f32, tag="outS")

            with tc.tile_pool(name="ps_pre", bufs=2, space="PSUM") as pre:
                # ---- GroupNorm stats ----
                st6 = sp_.tile([C, NBN, 6], f32, tag="st6")
                for i in range(NBN):
                    nc.vector.bn_stats(out=st6[:, i, :], in_=xS[:, i * BNC:(i + 1) * BNC])
                mv = sp_.tile([C, 2], f32, tag="mv")
                nc.vector.bn_aggr(out=mv[:], in_=st6[:])
                # mv col1 <- mean^2 + var = E[x^2] (in place)
                nc.vector.scalar_tensor_tensor(out=mv[:, 1:2], in0=mv[:, 0:1],
                                               scalar=mv[:, 0:1], in1=mv[:, 1:2],
                                               op0=OP.mult, op1=OP.add)
                # cross-partition group reduce: gstats[c,:] = [gmean, gEx2] of c's group
                gps = pre.tile([C, 2], f32, tag="gstats")
                nc.tensor.matmul(gps[:], mS[:], mv[:], start=True, stop=True)
                gst = sp_.tile([C, 2], f32, tag="gst")
                nc.vector.tensor_copy(gst[:], gps[:])
                # xv = eps + gEx2 - gmean^2  (group variance + eps)
                i32 = mybir.dt.int32
                gv = sp_.tile([C, 1], f32, tag="gv")
                nc.vector.scalar_tensor_tensor(out=gv[:], in0=gst[:, 0:1],
                                               scalar=gst[:, 0:1], in1=gst[:, 1:2],
                                               op0=OP.mult, op1=OP.subtract)
                xv = sp_.tile([C, 1], f32, tag="xv")
                nc.vector.tensor_tensor(out=xv[:], in0=epsT[:], in1=gv[:], op=OP.subtract)
                magicT = cp.tile([C, 1], i32, tag="magic")
                nc.vector.memset(magicT[:], 0x5F3759DF)
                yh = sp_.tile([C, 1], i32, tag="yh")
                nc.vector.tensor_scalar(out=yh[:], in0=xv[:].bitcast(i32), scalar1=1,
                                        scalar2=None, op0=OP.logical_shift_right)
                nc.vector.tensor_tensor(out=yh[:], in0=magicT[:], in1=yh[:], op=OP.subtract)
                inv = sp_.tile([C, 1], f32, tag="inv")
                nc.vector.tensor_copy(inv[:], yh[:].bitcast(f32))
                tN = sp_.tile([C, 1], f32, tag="tN")
                for _ in range(2):
                    nc.vector.tensor_tensor(out=tN[:], in0=inv[:], in1=inv[:], op=OP.mult)
                    nc.vector.tensor_tensor(out=tN[:], in0=tN[:], in1=xv[:], op=OP.mult)
                    nc.vector.tensor_scalar(out=tN[:], in0=tN[:], scalar1=-0.5,
                                            scalar2=1.5, op0=OP.mult, op1=OP.add)
                    nc.vector.tensor_tensor(out=inv[:], in0=inv[:], in1=tN[:], op=OP.mult)
                aT = sp_.tile([C, 1], f32, tag="aT")
                nc.vector.tensor_tensor(out=aT[:], in0=bS[:, 3:4], in1=inv[:], op=OP.mult)
                bT = sp_.tile([C, 1], f32, tag="bT")
                nc.vector.tensor_tensor(out=bT[:], in0=gst[:, 0:1], in1=aT[:], op=OP.mult)
                nc.vector.tensor_tensor(out=bT[:], in0=bS[:, 4:5], in1=bT[:], op=OP.subtract)
                # h = a*x + b  (rounded to f32r), chunked so projections can
                # start before the whole affine finishes
                for c in range(N // BNC):
                    nc.vector.tensor_scalar(out=hR[:, c * BNC:(c + 1) * BNC],
                                            in0=xS[:, c * BNC:(c + 1) * BNC],
                                            scalar1=aT[:], scalar2=bT[:],
                                            op0=OP.mult, op1=OP.add)

                # ---- projections ----
                hB = bp.tile([C, N], bf16, tag="hB")
                for c in range(N // BNC):
                    nc.vector.tensor_copy(hB[:, c * BNC:(c + 1) * BNC],
                                          hR[:, c * BNC:(c + 1) * BNC].bitcast(f32))
                for c in range(NPC):
                    qp = pre.tile([C, PCW], f32, tag="qp")
                    nc.tensor.matmul(qp[:], wR[:, 0:C], hR[:, PCW * c:PCW * (c + 1)],
                                     start=True, stop=True)
                    nc.scalar.activation(out=qR[:, PCW * c:PCW * (c + 1)], in_=qp[:],
                                         func=AF.Identity, bias=bS[:, 0:1], scale=1.0)
                for c in range(N // 512):
                    kp = pre.tile([C, 512], f32, tag="kp")
                    nc.tensor.matmul(kp[:], wR[:, C:2 * C], hR[:, 512 * c:512 * (c + 1)],
                                     start=True, stop=True)
                    nc.scalar.activation(out=kR[:, 512 * c:512 * (c + 1)], in_=kp[:],
                                         func=AF.Identity, bias=bS[:, 1:2], scale=1.0)
                for g in range(N // 512):
                    vtp = pre.tile([C, 512], f32, tag="vtp")
                    for j4 in range(4):
                        tj = 4 * g + j4
                        nc.tensor.matmul(vtp[:, 128 * j4:128 * (j4 + 1)],
                                         hB[:, 128 * tj:128 * (tj + 1)], wvB[:],
                                         start=True, stop=True)
                    nc.vector.tensor_copy(vTR[:, 512 * g:512 * (g + 1)], vtp[:])


            # ---- attention ----
            # sT double-buffered at STW wide (one exp op per tile). PE runs in
            # program order, so PV/ones for tile t are emitted one tile late:
            # while exp(t) runs on ACT, PE issues QK(t+1) instead of stalling.
            STW = min(1024, ICW)   # sT tile / exp chunk width
            NST = ICW // STW
            MMW = min(512, STW)    # matmul free-dim chunk
            NMM = STW // MMW
            NPAIR = NJT // 2  # rowsum matmuls run on pairwise P-sums (DVE adds)
            with tc.tile_pool(name="ps_sT", bufs=2, space="PSUM") as psT, \
                 tc.tile_pool(name="ps_h2", bufs=1, space="PSUM") as ph2, \
                 tc.tile_pool(name="ps_rs", bufs=1, space="PSUM") as prs:
                acc = {}        # ic -> (h2p, rsp)
                pend_pv = None  # (ic, odd t, Ppair) awaiting PV emission
                pend_ones = None  # (ic, pair_idx, Ps2) awaiting ones-MM emission

                def emit_pv(p):
                    # fp8 DoubleRow: one matmul contracts the pair of j-tiles
                    # (tp-1, tp); called only at odd tp.
                    icp, tp, Ppair = p
                    h2p = acc[icp][0]
                    pi = tp // 2
                    vpair = vTR[:, 256 * pi:256 * (pi + 1)].rearrange(
                        "p (two c) -> p two c", two=2)
                    for m in range(NMM):
                        nc.tensor.matmul(
                            h2p[:, m * MMW:(m + 1) * MMW], vpair,
                            Ppair[:, :, m * MMW:(m + 1) * MMW],
                            start=(pi == 0), stop=(pi == NJT // 2 - 1),
                            perf_mode=mybir.MatmulPerfMode.DoubleRow)

                def emit_ones(p, first=None, last=None, fp8=False):
                    icp, pi, Ps2p = p
                    rsp = acc[icp][1]
                    st = first if first is not None else (pi == 0)
                    sp2 = last if last is not None else False
                    lhs = onesF8[:] if fp8 else onesR[:]
                    for m in range(NMM):
                        nc.tensor.matmul(
                            rsp[:, m * MMW:(m + 1) * MMW], lhs,
                            Ps2p[:, m * MMW:(m + 1) * MMW],
                            start=st, stop=sp2)

                def finish_pass(ic):
                    h2p, rsp = acc[ic]
                    FCW = min(512, ICW)
                    for fc in range(ICW // FCW):
                        sl_i = slice(ic * ICW + fc * FCW, ic * ICW + (fc + 1) * FCW)
                        sl_f = slice(fc * FCW, (fc + 1) * FCW)
                        recipB = sp_.tile([C, FCW], f32, tag="recipB")
                        nc.vector.reciprocal_approx_fast(out=recipB[:], in_=rsp[:, sl_f])
                        nc.vector.tensor_tensor(out=h2nR[:, sl_i], in0=h2p[:, sl_f],
                                                in1=recipB[:], op=OP.mult)

                for ic in range(NIC):
                    acc[ic] = (ph2.tile([C, ICW], f32, tag="h2u", name=f"h2u{ic}"),
                               prs.tile([C, ICW], f32, tag="rs", name=f"rs{ic}"))
                    Ppair = None
                    for t in range(NJT):
                        for c2 in range(NST):
                            i0 = ic * ICW + c2 * STW
                            sT = psT.tile([C, STW], f32, tag="sT")
                            for m in range(NMM):
                                nc.tensor.matmul(
                                    sT[:, m * MMW:(m + 1) * MMW],
                                    kR[:, 128 * t:128 * (t + 1)],
                                    qR[:, i0 + m * MMW:i0 + (m + 1) * MMW],
                                    start=True, stop=True)
                            if t % 2 == 0:
                                Ppair = pP.tile([C, 2, STW], f8, tag="P",
                                                name=f"P{ic}_{t}")
                            nc.scalar.activation(out=Ppair[:, t % 2, :], in_=sT[:],
                                                 func=AF.Exp, scale=SCALE)
                            if t % 2 == 1:
                                pend_pv = (ic, t, Ppair)
                                continue
                            if pend_pv is not None:
                                emit_pv(pend_pv)
                                tp = pend_pv[1]
                                Pp = pend_pv[2]
                                if tp == NJT - 1:
                                    # tail of the pass: direct fp8 ones-MMs so
                                    # the rowsum doesn't wait on a DVE pair-add
                                    if pend_ones is not None:
                                        emit_ones(pend_ones)
                                        pend_ones = None
                                    emit_ones((pend_pv[0], -1, Pp[:, 0, :]),
                                              first=False, last=False, fp8=True)
                                    emit_ones((pend_pv[0], -1, Pp[:, 1, :]),
                                              first=False, last=True, fp8=True)
                                    finish_pass(pend_pv[0])
                                else:
                                    # DVE pair-sum of the two P slices just used
                                    Ps2 = pP.tile([C, STW], f32r, tag="Ps2")
                                    nc.vector.tensor_tensor(
                                        out=Ps2[:], in0=Pp[:, 0, :],
                                        in1=Pp[:, 1, :], op=OP.add)
                                    if pend_ones is not None:
                                        emit_ones(pend_ones)
                                    pend_ones = (pend_pv[0], tp // 2, Ps2)
                                pend_pv = None
                emit_pv(pend_pv)
                tp = pend_pv[1]
                Pp = pend_pv[2]
                if pend_ones is not None:
                    emit_ones(pend_ones)
                    pend_ones = None
                emit_ones((pend_pv[0], -1, Pp[:, 0, :]), first=False, last=False,
                          fp8=True)
                emit_ones((pend_pv[0], -1, Pp[:, 1, :]), first=False, last=True,
                          fp8=True)
                finish_pass(pend_pv[0])

            # ---- out projection + bias + residual ----
            with tc.tile_pool(name="ps_ep", bufs=2, space="PSUM") as pep:
                for c in range(NPC):
                    pop = pep.tile([C, PCW], f32, tag="pop")
                    nc.tensor.matmul(pop[:], wR[:, 3 * C:4 * C],
                                     h2nR[:, PCW * c:PCW * (c + 1)], start=True, stop=True)
                    nc.vector.scalar_tensor_tensor(
                        out=outS[:, PCW * c:PCW * (c + 1)], in0=pop[:], scalar=bS[:, 2:3],
                        in1=xS[:, PCW * c:PCW * (c + 1)], op0=OP.add, op1=OP.add)
                    nc.sync.dma_start(o_d[:, PCW * c:PCW * (c + 1)],
                                      outS[:, PCW * c:PCW * (c + 1)])
            if _loop is not None:
                _loop.__exit__(None, None, None)

    nc.compile()
    return nc


def host_inputs(x, gn_w, gn_b, w_qkv, b_qkv, w_out, b_out):
    """Build the 8 per-core input maps from the full problem inputs."""
    x = np.asarray(x, dtype=np.float32)
    B, _, N = x.shape
    S = N // 2
    w_qkv = np.asarray(w_qkv, np.float32)
    w_out = np.asarray(w_out, np.float32)
    b_qkv = np.asarray(b_qkv, np.float32)
    b_out = np.asarray(b_out, np.float32)
    gn_w = np.asarray(gn_w, np.float32)
    gn_b = np.asarray(gn_b, np.float32)

    wcat = np.concatenate(
        [w_qkv[0:C].T, w_qkv[C:2 * C].T, w_qkv[2 * C:3 * C].T, w_out.T],
        axis=1).astype(np.float32)                      # [C, 4C], each [c_in, c_out]
    gidx = np.arange(C) // GS
    gmask = (gidx[:, None] == gidx[None, :]).astype(np.float32) / GS
    b_eff = b_out + w_out @ b_qkv[2 * C:3 * C]
    bcat = np.stack([b_qkv[0:C], b_qkv[C:2 * C], b_eff, gn_w, gn_b], axis=1)
    bcat = np.ascontiguousarray(bcat, np.float32)       # [C, 5]

    in_maps = []
    for core in range(N_CORES):
        b, half = divmod(core, 2)
        xb = np.roll(x[b], -half * S, axis=1)
        in_maps.append({"x": np.ascontiguousarray(xb), "wcat": wcat,
                        "gmask": gmask, "bcat": bcat})
    return in_maps


_NC_CACHE = {}
_RUNNER_CACHE = {}


def _make_runner(nc):
    """Compile-once runner: replicates bass2jax.run_bass_via_pjrt but keeps the
    jitted sharded callable so repeat executions skip recompilation."""
    import jax
    import concourse.mybir as mybir
    from jax.sharding import Mesh, PartitionSpec
    from jax.experimental.shard_map import shard_map
    from concourse.bass2jax import (_bass_exec_p, install_neuronx_cc_hook,
                                    partition_id_tensor)

    install_neuronx_cc_hook()
    partition_name = nc.partition_id_tensor.name if nc.partition_id_tensor else None
    in_names, out_names, out_avals, zero_shapes = [], [], [], []
    for alloc in nc.m.functions[0].allocations:
        if not isinstance(alloc, mybir.MemoryLocationSet):
            continue
        name = alloc.memorylocations[0].name
        if alloc.kind == "ExternalInput":
            if name == partition_name:
                continue
            in_names.append(name)
        elif alloc.kind == "ExternalOutput":
            out_names.append(name)
            shape = tuple(alloc.tensor_shape)
            dtype = mybir.dt.np(alloc.dtype)
            out_avals.append(jax.core.ShapedArray(shape, dtype))
            zero_shapes.append((shape, dtype))
    n_params = len(in_names)
    all_names = in_names + out_names
    if partition_name is not None:
        all_names = all_names + [partition_name]
    donate = tuple(range(n_params, n_params + len(out_names)))

    def _body(*args):
        operands = list(args)
        if partition_name is not None:
            operands.append(partition_id_tensor())
        return tuple(_bass_exec_p.bind(
            *operands, out_avals=tuple(out_avals), in_names=tuple(all_names),
            out_names=tuple(out_names), lowering_input_output_aliases=(),
            sim_require_finite=True, sim_require_nnan=True, nc=nc))

    devices = jax.devices()[:N_CORES]
    mesh = Mesh(np.asarray(devices), ("core",))
    specs = (PartitionSpec("core"),)
    sharded = jax.jit(
        shard_map(_body, mesh=mesh,
                  in_specs=specs * (n_params + len(out_names)),
                  out_specs=specs * len(out_names), check_rep=False),
        donate_argnums=donate, keep_unused=True)

    def run(in_maps):
        concat_in = [np.concatenate([np.asarray(m[nm]) for m in in_maps], axis=0)
                     for nm in in_names]
        concat_zeros = [np.zeros((N_CORES * s[0], *s[1:]), d) for s, d in zero_shapes]
        out_arrs = sharded(*concat_in, *concat_zeros)
        out_arrs = [np.asarray(a) for a in out_arrs]
        return [{nm: out_arrs[i].reshape(N_CORES, *out_avals[i].shape)[c]
                 for i, nm in enumerate(out_names)} for c in range(N_CORES)]

    return run


def get_runner(N=4096):
    if N not in _RUNNER_CACHE:
        if N not in _NC_CACHE:
            _NC_CACHE[N] = build(N)
        _RUNNER_CACHE[N] = _make_runner(_NC_CACHE[N])
    return _RUNNER_CACHE[N]


def kernel(x, gn_w, gn_b, w_qkv, b_qkv, w_out, b_out):
    x = np.asarray(x, dtype=np.float32)
    B, _, N = x.shape
    S = N // 2
    run = get_runner(N)
    in_maps = host_inputs(x, gn_w, gn_b, w_qkv, b_qkv, w_out, b_out)
    results = run(in_maps)
    out = np.empty((B, C, N), dtype=np.float32)
    for core in range(N_CORES):
        b, half = divmod(core, 2)
        out[b, :, half * S:(half + 1) * S] = results[core]["out"]
    return out


# revision 39
# speedup vs baseline: 7974.0556x; 1.0299x over previous
"""AttentionBlock (GroupNorm -> QKV -> full attention -> out-proj + residual)
for B=4, C=128, N=4096 on 8 Trainium2 NeuronCores.

Sharding: 8 cores = 4 batches x 2 query-slabs of N/2. Every core runs the
same program; the host rolls each core's x so its query slab is always
columns [0, N/2).

Matmuls run in float32r (fp32 data, the PE rounds to ~tf32 -> ~1e-4 rel err
at full speed); the v-transpose projection runs in bf16. Scores are computed
transposed [j, i] so softmax's exp feeds the PV matmul with no transposes.
Row sums come from all-ones matmuls over DVE-pairsummed P tiles, accumulated
in PSUM alongside PV with every partition holding the same row-sum; the
softmax normalization is then one reciprocal_approx_fast + multiply per
pass. PE executes in program order, so PV/ones for tile t are emitted one
tile late (software pipelining against the ACT exp).
"""

import math
import sys

if "/opt/trn_rl_repo" not in sys.path:
    sys.path.insert(0, "/opt/trn_rl_repo")

import numpy as np

C = 128
G = 8
GS = C // G  # channels per group
EPS = 1e-5
N_CORES = 8


def build(N=4096, repeat=1):
    """Build the per-core Bass program. Returns the compiled Bacc module."""
    import concourse.bacc as bacc
    import concourse.bass as bass
    import concourse.mybir as mybir
    import concourse.tile as tile

    f32 = mybir.dt.float32
    f32r = mybir.dt.float32r
    AF = mybir.ActivationFunctionType
    OP = mybir.AluOpType

    S = N // 2           # query slab width per core
    ICW = min(1024, S)   # i-chunk width (one PV/rowsum accumulation pass)
    NIC = S // ICW       # number of i-chunk passes
    NJT = N // 128       # number of j (key) tiles
    BNC = min(512, N)    # bn_stats chunk
    NBN = N // BNC
    PCW = min(512, S)    # projection/epilogue chunk width for slab-sized tensors
    NPC = S // PCW
    SCALE = 1.0 / math.sqrt(C)

    nc = bacc.Bacc("TRN2", target_bir_lowering=False, debug=False)

    x_d = nc.dram_tensor("x", [C, N], f32, kind="ExternalInput").ap()
    w_d = nc.dram_tensor("wcat", [C, 4 * C], f32, kind="ExternalInput").ap()
    m_d = nc.dram_tensor("gmask", [C, C], f32, kind="ExternalInput").ap()
    b_d = nc.dram_tensor("bcat", [C, 5], f32, kind="ExternalInput").ap()
    o_d = nc.dram_tensor("out", [C, S], f32, kind="ExternalOutput").ap()

    with tile.TileContext(nc) as tc:
        with tc.tile_pool(name="consts", bufs=1) as cp, \
             tc.tile_pool(name="big", bufs=1) as bp, \
             tc.tile_pool(name="small", bufs=3) as sp_, \
             tc.tile_pool(name="pP", bufs=6) as pP:
            _loop = tc.For_i(0, repeat, 1) if repeat > 1 else None
            if _loop is not None:
                _loop.__enter__()

            # ---- loads + constants ----
            xS = bp.tile([C, N], f32, tag="x")
            for dc in range(NBN):
                nc.sync.dma_start(xS[:, dc * BNC:(dc + 1) * BNC],
                                  x_d[:, dc * BNC:(dc + 1) * BNC])
            wS = cp.tile([C, 4 * C], f32, tag="w")
            nc.sync.dma_start(wS[:], w_d[:])
            wR = cp.tile([C, 4 * C], f32r, tag="wr")
            nc.vector.tensor_copy(wR[:], wS[:])
            mS = cp.tile([C, C], f32, tag="gmask")
            nc.sync.dma_start(mS[:], m_d[:])
            bS = cp.tile([C, 5], f32, tag="bcat")
            nc.sync.dma_start(bS[:], b_d[:])
            onesS = cp.tile([C, C], f32, tag="ones")
            nc.vector.memset(onesS[:], 1.0)
            onesR = cp.tile([C, C], f32r, tag="onesr")
            nc.vector.tensor_copy(onesR[:], onesS[:])
            epsT = cp.tile([C, 1], f32, tag="eps")
            nc.vector.memset(epsT[:], EPS)
            bf16 = mybir.dt.bfloat16
            f8 = mybir.dt.float8e4
            wvB = cp.tile([C, C], bf16, tag="wvB")
            nc.vector.tensor_copy(wvB[:], wS[:, 2 * C:3 * C])
            onesF8 = cp.tile([C, C], f8, tag="onesf8")
            nc.vector.tensor_copy(onesF8[:], onesS[:])

            hR = bp.tile([C, N], f32r, tag="h")
            qtR = bp.tile([C, S], f32r, tag="qt")
            vTR = bp.tile([C, N], f8, tag="vT")
            h2nR = bp.tile([C, S], f32r, tag="h2n")
            outS = bp.tile([C, S], f32, tag="outS")

            with tc.tile_pool(name="ps_pre", bufs=2, space="PSUM") as pre:
                # ---- GroupNorm stats ----
                st6 = sp_.tile([C, NBN, 6], f32, tag="st6")
                for i in range(NBN):
                    nc.vector.bn_stats(out=st6[:, i, :], in_=xS[:, i * BNC:(i + 1) * BNC])
                mv = sp_.tile([C, 2], f32, tag="mv")
                nc.vector.bn_aggr(out=mv[:], in_=st6[:])
                # mv col1 <- mean^2 + var = E[x^2] (in place)
                nc.vector.scalar_tensor_tensor(out=mv[:, 1:2], in0=mv[:, 0:1],
                                               scalar=mv[:, 0:1], in1=mv[:, 1:2],
                                               op0=OP.mult, op1=OP.add)
                # cross-partition group reduce: gstats[c,:] = [gmean, gEx2] of c's group
                gps = pre.tile([C, 2], f32, tag="gstats")
                nc.tensor.matmul(gps[:], mS[:], mv[:], start=True, stop=True)
                gst = sp_.tile([C, 2], f32, tag="gst")
                nc.vector.tensor_copy(gst[:], gps[:])
                # xv = eps + gEx2 - gmean^2  (group variance + eps)
                i32 = mybir.dt.int32
                gv = sp_.tile([C, 1], f32, tag="gv")
                nc.vector.scalar_tensor_tensor(out=gv[:], in0=gst[:, 0:1],
                                               scalar=gst[:, 0:1], in1=gst[:, 1:2],
                                               op0=OP.mult, op1=OP.subtract)
                xv = sp_.tile([C, 1], f32, tag="xv")
                nc.vector.tensor_tensor(out=xv[:], in0=epsT[:], in1=gv[:], op=OP.subtract)
                magicT = cp.tile([C, 1], i32, tag="magic")
                nc.vector.memset(magicT[:], 0x5F3759DF)
                yh = sp_.tile([C, 1], i32, tag="yh")
                nc.vector.tensor_scalar(out=yh[:], in0=xv[:].bitcast(i32), scalar1=1,
                                        scalar2=None, op0=OP.logical_shift_right)
                nc.vector.tensor_tensor(out=yh[:], in0=magicT[:], in1=yh[:], op=OP.subtract)
                inv = sp_.tile([C, 1], f32, tag="inv")
                nc.vector.tensor_copy(inv[:], yh[:].bitcast(f32))
                tN = sp_.tile([C, 1], f32, tag="tN")
                for _ in range(2):
                    nc.vector.tensor_tensor(out=tN[:], in0=inv[:], in1=inv[:], op=OP.mult)
                    nc.vector.tensor_tensor(out=tN[:], in0=tN[:], in1=xv[:], op=OP.mult)
                    nc.vector.tensor_scalar(out=tN[:], in0=tN[:], scalar1=-0.5,
                                            scalar2=1.5, op0=OP.mult, op1=OP.add)
                    nc.vector.tensor_tensor(out=inv[:], in0=inv[:], in1=tN[:], op=OP.mult)
                aT = sp_.tile([C, 1], f32, tag="aT")
                nc.vector.tensor_tensor(out=aT[:], in0=bS[:, 3:4], in1=inv[:], op=OP.mult)
                bT = sp_.tile([C, 1], f32, tag="bT")
                nc.vector.tensor_tensor(out=bT[:], in0=gst[:, 0:1], in1=aT[:], op=OP.mult)
                nc.vector.tensor_tensor(out=bT[:], in0=bS[:, 4:5], in1=bT[:], op=OP.subtract)
                # h = a*x + b  (rounded to f32r), chunked so projections can
                # start before the whole affine finishes
                for c in range(N // BNC):
                    nc.vector.tensor_scalar(out=hR[:, c * BNC:(c + 1) * BNC],
                                            in0=xS[:, c * BNC:(c + 1) * BNC],
                                            scalar1=aT[:], scalar2=bT[:],
                                            op0=OP.mult, op1=OP.add)

                # ---- projections ----
                hB = bp.tile([C, N], bf16, tag="hB")
                for c in range(N // BNC):
                    nc.gpsimd.tensor_copy(out=hB[:, c * BNC:(c + 1) * BNC],
                                           in_=hR[:, c * BNC:(c + 1) * BNC].bitcast(f32))
                for c in range(NPC):
                    # qt = (w_q^T w_k)^T h + w_k^T b_q;  scores = h^T qt
                    qtp = pre.tile([C, PCW], f32, tag="qtp")
                    nc.tensor.matmul(qtp[:], wR[:, 0:C], hR[:, PCW * c:PCW * (c + 1)],
                                     start=True, stop=True)
                    nc.scalar.activation(out=qtR[:, PCW * c:PCW * (c + 1)], in_=qtp[:],
                                         func=AF.Identity, bias=bS[:, 0:1], scale=1.0)
                for g in range(N // 512):
                    vtp = pre.tile([C, 512], f32, tag="vtp")
                    for j4 in range(4):
                        tj = 4 * g + j4
                        nc.tensor.matmul(vtp[:, 128 * j4:128 * (j4 + 1)],
                                         hB[:, 128 * tj:128 * (tj + 1)], wvB[:],
                                         start=True, stop=True)
                    nc.vector.tensor_copy(vTR[:, 512 * g:512 * (g + 1)], vtp[:])


            # ---- attention ----
            # sT double-buffered at STW wide (one exp op per tile). PE runs in
            # program order, so PV/ones for tile t are emitted one tile late:
            # while exp(t) runs on ACT, PE issues QK(t+1) instead of stalling.
            STW = min(1024, ICW)   # sT tile / exp chunk width
            NST = ICW // STW
            MMW = min(512, STW)    # matmul free-dim chunk
            NMM = STW // MMW
            NPAIR = NJT // 2  # rowsum matmuls run on pairwise P-sums (DVE adds)
            with tc.tile_pool(name="ps_rs", bufs=1, space="PSUM") as prs, \
                 tc.tile_pool(name="ps_h2", bufs=1, space="PSUM") as ph2, \
                 tc.tile_pool(name="ps_sT", bufs=2, space="PSUM") as psT:
                acc = {}        # ic -> (h2p, rsp)
                pend_pv = None  # (ic, odd t, Ppair) awaiting PV emission
                pend_ones = None  # (ic, pair_idx, Ps2) awaiting ones-MM emission

                def emit_pv(p):
                    # fp8 DoubleRow: one matmul contracts the pair of j-tiles
                    # (tp-1, tp); called only at odd tp.
                    icp, tp, Ppair = p
                    h2p = acc[icp][0]
                    pi = tp // 2
                    vpair = vTR[:, 256 * pi:256 * (pi + 1)].rearrange(
                        "p (two c) -> p two c", two=2)
                    for m in range(NMM):
                        nc.tensor.matmul(
                            h2p[:, m * MMW:(m + 1) * MMW], vpair,
                            Ppair[:, :, m * MMW:(m + 1) * MMW],
                            start=(pi == 0), stop=(pi == NJT // 2 - 1),
                            perf_mode=mybir.MatmulPerfMode.DoubleRow)

                def emit_ones(p, first=None, last=None, fp8=False):
                    icp, pi, Ps2p = p
                    rsp = acc[icp][1]
                    st = first if first is not None else (pi == 0)
                    sp2 = last if last is not None else False
                    lhs = onesF8[:] if fp8 else onesR[:]
                    for m in range(NMM):
                        nc.tensor.matmul(
                            rsp[:, m * MMW:(m + 1) * MMW], lhs,
                            Ps2p[:, m * MMW:(m + 1) * MMW],
                            start=st, stop=sp2)

                def finish_pass(ic):
                    h2p, rsp = acc[ic]
                    FCW = min(512, ICW)
                    for fc in range(ICW // FCW):
                        sl_i = slice(ic * ICW + fc * FCW, ic * ICW + (fc + 1) * FCW)
                        sl_f = slice(fc * FCW, (fc + 1) * FCW)
                        recipB = sp_.tile([C, FCW], f32, tag="recipB")
                        nc.vector.reciprocal_approx_fast(out=recipB[:], in_=rsp[:, sl_f])
                        nc.vector.tensor_tensor(out=h2nR[:, sl_i], in0=h2p[:, sl_f],
                                                in1=recipB[:], op=OP.mult)

                for ic in range(NIC):
                    acc[ic] = (ph2.tile([C, ICW], f32, tag="h2u", name=f"h2u{ic}"),
                               prs.tile([C, ICW], f32, tag="rs", name=f"rs{ic}"))
                    Ppair = None
                    for t in range(NJT):
                        for c2 in range(NST):
                            i0 = ic * ICW + c2 * STW
                            sT = psT.tile([C, STW], f32, tag="sT")
                            for m in range(NMM):
                                nc.tensor.matmul(
                                    sT[:, m * MMW:(m + 1) * MMW],
                                    hR[:, 128 * t:128 * (t + 1)],
                                    qtR[:, i0 + m * MMW:i0 + (m + 1) * MMW],
                                    start=True, stop=True)
                            if t % 2 == 0:
                                Ppair = pP.tile([C, 2, STW], f8, tag="P",
                                                name=f"P{ic}_{t}")
                            nc.scalar.activation(out=Ppair[:, t % 2, :], in_=sT[:],
                                                 func=AF.Exp, scale=SCALE)
                            if t % 2 == 1:
                                pend_pv = (ic, t, Ppair)
                                continue
                            if pend_pv is not None:
                                emit_pv(pend_pv)
                                tp = pend_pv[1]
                                Pp = pend_pv[2]
                                if tp == NJT - 1:
                                    # tail of the pass: direct fp8 ones-MMs so
                                    # the rowsum doesn't wait on a DVE pair-add
                                    if pend_ones is not None:
                                        emit_ones(pend_ones)
                                        pend_ones = None
                                    emit_ones((pend_pv[0], -1, Pp[:, 0, :]),
                                              first=False, last=False, fp8=True)
                                    emit_ones((pend_pv[0], -1, Pp[:, 1, :]),
                                              first=False, last=True, fp8=True)
                                    finish_pass(pend_pv[0])
                                else:
                                    # DVE pair-sum of the two P slices just used
                                    Ps2 = pP.tile([C, STW], f32r, tag="Ps2")
                                    nc.vector.tensor_tensor(
                                        out=Ps2[:], in0=Pp[:, 0, :],
                                        in1=Pp[:, 1, :], op=OP.add)
                                    if pend_ones is not None:
                                        emit_ones(pend_ones)
                                    pend_ones = (pend_pv[0], tp // 2, Ps2)
                                pend_pv = None
                emit_pv(pend_pv)
                tp = pend_pv[1]
                Pp = pend_pv[2]
                if pend_ones is not None:
                    emit_ones(pend_ones)
                    pend_ones = None
                emit_ones((pend_pv[0], -1, Pp[:, 0, :]), first=False, last=False,
                          fp8=True)
                emit_ones((pend_pv[0], -1, Pp[:, 1, :]), first=False, last=True,
                          fp8=True)
                finish_pass(pend_pv[0])

            # ---- out projection + bias + residual ----
            with tc.tile_pool(name="ps_ep", bufs=2, space="PSUM") as pep:
                for c in range(NPC):
                    pop = pep.tile([C, PCW], f32, tag="pop")
                    nc.tensor.matmul(pop[:], wR[:, 3 * C:4 * C],
                                     h2nR[:, PCW * c:PCW * (c + 1)], start=True, stop=True)
                    nc.vector.scalar_tensor_tensor(
                        out=outS[:, PCW * c:PCW * (c + 1)], in0=pop[:], scalar=bS[:, 2:3],
                        in1=xS[:, PCW * c:PCW * (c + 1)], op0=OP.add, op1=OP.add)
                    nc.sync.dma_start(o_d[:, PCW * c:PCW * (c + 1)],
                                      outS[:, PCW * c:PCW * (c + 1)])
            if _loop is not None:
                _loop.__exit__(None, None, None)

    nc.compile()
    return nc


def host_inputs(x, gn_w, gn_b, w_qkv, b_qkv, w_out, b_out):
    """Build the 8 per-core input maps from the full problem inputs."""
    x = np.asarray(x, dtype=np.float32)
    B, _, N = x.shape
    S = N // 2
    w_qkv = np.asarray(w_qkv, np.float32)
    w_out = np.asarray(w_out, np.float32)
    b_qkv = np.asarray(b_qkv, np.float32)
    b_out = np.asarray(b_out, np.float32)
    gn_w = np.asarray(gn_w, np.float32)
    gn_b = np.asarray(gn_b, np.float32)

    # scores = h^T (w_q^T w_k) h + h^T (w_k^T b_q); the k bias is
    # softmax-invariant and dropped, q/k are never materialized on device.
    M = w_qkv[0:C].T @ w_qkv[C:2 * C]
    wcat = np.concatenate(
        [M, np.zeros((C, C), np.float32), w_qkv[2 * C:3 * C].T, w_out.T],
        axis=1).astype(np.float32)   # [C, 4C]: [M, unused, w_v^T, w_out^T]
    gidx = np.arange(C) // GS
    gmask = (gidx[:, None] == gidx[None, :]).astype(np.float32) / GS
    b_eff = b_out + w_out @ b_qkv[2 * C:3 * C]
    bqt = w_qkv[C:2 * C].T @ b_qkv[0:C]
    bcat = np.stack([bqt, b_qkv[C:2 * C], b_eff, gn_w, gn_b], axis=1)
    bcat = np.ascontiguousarray(bcat, np.float32)       # [C, 5]

    in_maps = []
    for core in range(N_CORES):
        b, half = divmod(core, 2)
        xb = np.roll(x[b], -half * S, axis=1)
        in_maps.append({"x": np.ascontiguousarray(xb), "wcat": wcat,
                        "gmask": gmask, "bcat": bcat})
    return in_maps


_NC_CACHE = {}
_RUNNER_CACHE = {}


def _make_runner(nc):
    """Compile-once runner: replicates bass2jax.run_bass_via_pjrt but keeps the
    jitted sharded callable so repeat executions skip recompilation."""
    import jax
    import concourse.mybir as mybir
    from jax.sharding import Mesh, PartitionSpec
    from jax.experimental.shard_map import shard_map
    from concourse.bass2jax import (_bass_exec_p, install_neuronx_cc_hook,
                                    partition_id_tensor)

    install_neuronx_cc_hook()
    partition_name = nc.partition_id_tensor.name if nc.partition_id_tensor else None
    in_names, out_names, out_avals, zero_shapes = [], [], [], []
    for alloc in nc.m.functions[0].allocations:
        if not isinstance(alloc, mybir.MemoryLocationSet):
            continue
        name = alloc.memorylocations[0].name
        if alloc.kind == "ExternalInput":
            if name == partition_name:
                continue
            in_names.append(name)
        elif alloc.kind == "ExternalOutput":
            out_names.append(name)
            shape = tuple(alloc.tensor_shape)
            dtype = mybir.dt.np(alloc.dtype)
            out_avals.append(jax.core.ShapedArray(shape, dtype))
            zero_shapes.append((shape, dtype))
    n_params = len(in_names)
    all_names = in_names + out_names
    if partition_name is not None:
        all_names = all_names + [partition_name]
    donate = tuple(range(n_params, n_params + len(out_names)))

    def _body(*args):
        operands = list(args)
        if partition_name is not None:
            operands.append(partition_id_tensor())
        return tuple(_bass_exec_p.bind(
            *operands, out_avals=tuple(out_avals), in_names=tuple(all_names),
            out_names=tuple(out_names), lowering_input_output_aliases=(),
            sim_require_finite=True, sim_require_nnan=True, nc=nc))

    devices = jax.devices()[:N_CORES]
    mesh = Mesh(np.asarray(devices), ("core",))
    specs = (PartitionSpec("core"),)
    sharded = jax.jit(
        shard_map(_body, mesh=mesh,
                  in_specs=specs * (n_params + len(out_names)),
                  out_specs=specs * len(out_names), check_rep=False),
        donate_argnums=donate, keep_unused=True)

    def run(in_maps):
        concat_in = [np.concatenate([np.asarray(m[nm]) for m in in_maps], axis=0)
                     for nm in in_names]
        concat_zeros = [np.zeros((N_CORES * s[0], *s[1:]), d) for s, d in zero_shapes]
        out_arrs = sharded(*concat_in, *concat_zeros)
        out_arrs = [np.asarray(a) for a in out_arrs]
        return [{nm: out_arrs[i].reshape(N_CORES, *out_avals[i].shape)[c]
                 for i, nm in enumerate(out_names)} for c in range(N_CORES)]

    return run


def get_runner(N=4096):
    if N not in _RUNNER_CACHE:
        if N not in _NC_CACHE:
            _NC_CACHE[N] = build(N)
        _RUNNER_CACHE[N] = _make_runner(_NC_CACHE[N])
    return _RUNNER_CACHE[N]


def kernel(x, gn_w, gn_b, w_qkv, b_qkv, w_out, b_out):
    x = np.asarray(x, dtype=np.float32)
    B, _, N = x.shape
    S = N // 2
    run = get_runner(N)
    in_maps = host_inputs(x, gn_w, gn_b, w_qkv, b_qkv, w_out, b_out)
    results = run(in_maps)
    out = np.empty((B, C, N), dtype=np.float32)
    for core in range(N_CORES):
        b, half = divmod(core, 2)
        out[b, :, half * S:(half + 1) * S] = results[core]["out"]
    return out
